# revision 1
# baseline (speedup 1.0000x reference)
"""GQA attention layer (QKV proj + RoPE + softmax attention + out proj) on 8
Trainium2 NeuronCores.

Sharding: core c = (batch b = c//4, head-group g = c%4): 4 q heads + 1 kv
head per core, w_o row-parallel partial output in bf16; the host upcasts,
sums the 4 partials per batch and divides by the fp8 scaling factor (512).

Design highlights vs the original baseline (234 us -> 164 us):
- fp8 hi/lo DoubleRow matmuls for both projections: operands split into
  e4m3 hi + e5m2 lo (host-side for h/w_qkv/w_o, Pool-side for the attention
  output); each K=256 block runs as 3 DR matmuls (hi*hi + hi*lo + lo*hi) at
  0.75x the bf16 PE cost with ~bf16 accuracy. scores/attnV stay bf16
  (single-fp8 would blow the 2e-2 error budget).
- SCALE folded into the exp activation so q and k share one compact rope
  table; exp runs on Activation in [128, 1024] tiles; 1-2 tiles per head
  (~18% of keys) offload to DVE via a Schraudolph bitcast exp
  (int16(A*s + B) reinterpreted as bf16), weighted toward rounds where the
  PE has no filler work.
- softmax normalize: pacc copied off PSUM immediately (the psA bank recycles
  in one op), reciprocal on a partition-0 staged denominator (PSUM-sourced
  or partition-offset reciprocal inputs return garbage on HW), Pool
  partition_broadcast + Pool multiply replace the baseline's fp32 PE
  broadcast matmuls (everything SBUF-side is Pool-legal).
- schedule: two staggered attention-head generators with a filler queue
  (projections at 2x pump rate, V transposes, outproj tiles at 1/4 rate to
  cover whole q-tiles); junk warm-up matmuls bridge the initial DMA wait and
  the final norm chain so the PE p-state never drops mid-kernel; DMA order
  tuned so each k/v tile lands just before the attention wavefront needs it.
"""

import collections

import numpy as np
import ml_dtypes

B, S, HID = 2, 2048, 1024
NH, NKV, D = 16, 4, 64
SCALE = float(D ** -0.5)
NCORES = 8
TT = 512          # token tile
NTT = S // TT     # 4
KC = S // 128     # 16 key chunks
NCP = KC // 2     # 8 chunk pairs
OUT_SCALE = 1.0 / 512.0

_BF16 = ml_dtypes.bfloat16
_E4 = ml_dtypes.float8_e4m3
_E5 = ml_dtypes.float8_e5m2

_nc_cache = None


def _build_bass():
    import concourse.bass as bass
    import concourse.mybir as mybir
    import concourse.tile as tile
    from concourse import bacc
    from concourse.masks import make_identity

    BF = mybir.dt.bfloat16
    F32 = mybir.dt.float32
    I16 = mybir.dt.int16
    E4 = mybir.dt.float8e4
    E5 = mybir.dt.float8e5
    AF = mybir.ActivationFunctionType
    MULT = mybir.AluOpType.mult
    ADD = mybir.AluOpType.add
    SUB = mybir.AluOpType.subtract
    DR = mybir.MatmulPerfMode.DoubleRow
    # Schraudolph exp for offloaded tiles: bitcast(int16(s*A + B)) ~ exp(s)
    SCH_A = 184.66496280558537 * SCALE   # 128/ln2, scores carry 1/SCALE
    SCH_B = 16256.0 - 5.75 + 0.5         # bias center + truncation fix

    nc = bacc.Bacc()
    # h split hi/lo, packed [p, j, i, t]: h feature 256j + 128i + p
    hhi = nc.dram_tensor("hhi", (128, 4, 2, S), E4, kind="ExternalInput")
    hlo = nc.dram_tensor("hlo", (128, 4, 2, S), E5, kind="ExternalInput")
    # wqk split hi/lo, packed [p, rc, j, i, m]
    whi = nc.dram_tensor("whi", (128, 3, 4, 2, 128), E4, kind="ExternalInput")
    wlo = nc.dram_tensor("wlo", (128, 3, 4, 2, 128), E5, kind="ExternalInput")
    wohi = nc.dram_tensor("wohi", (128, 2, HID), E4, kind="ExternalInput")
    wolo = nc.dram_tensor("wolo", (128, 2, HID), E5, kind="ExternalInput")
    # shared q/k rope tables (/16); SCALE is applied by the exp activation
    cosd = nc.dram_tensor("cosd", (64, S), BF, kind="ExternalInput")
    sind = nc.dram_tensor("sind", (64, S), BF, kind="ExternalInput")
    out = nc.dram_tensor("out", (S, HID), BF, kind="ExternalOutput")

    with tile.TileContext(nc) as tc:
        with (
            tc.tile_pool(name="persist", bufs=1) as pp,
            tc.tile_pool(name="pbfp", bufs=3) as pbfp,
            tc.tile_pool(name="rope", bufs=3) as rp,
            tc.tile_pool(name="exps", bufs=6) as ep,
            tc.tile_pool(name="norm", bufs=4) as np_,
            tc.tile_pool(name="outsb", bufs=4) as op_,
        ):
            # ---- persistent SBUF tiles + input loads, kv-first order
            hhi_sb = pp.tile([128, 4, 2, S], E4, tag="hhi_sb")
            hlo_sb = pp.tile([128, 4, 2, S], E5, tag="hlo_sb")
            whi_sb = pp.tile([128, 3, 4, 2, 128], E4, tag="whi_sb")
            wlo_sb = pp.tile([128, 3, 4, 2, 128], E5, tag="wlo_sb")
            wohi_sb = pp.tile([128, 2, HID], E4, tag="wohi_sb")
            wolo_sb = pp.tile([128, 2, HID], E5, tag="wolo_sb")
            # [128, S]: rows 0:64 DMA'd, rows 64:128 duplicated on-device
            cos_sb = pp.tile([128, S], BF, tag="cos_sb")
            sin_sb = pp.tile([128, S], BF, tag="sin_sb")

            def h_slice(t0, t1):
                for hd, hs in ((hhi, hhi_sb), (hlo, hlo_sb)):
                    nc.sync.dma_start(hs[:, :, :, t0:t1], hd[:, :, :, t0:t1])

            nc.sync.dma_start(whi_sb[:, 2], whi[:, 2])
            nc.sync.dma_start(hhi_sb[:, 0:2, :, 0:TT], hhi[:, 0:2, :, 0:TT])
            nc.sync.dma_start(hhi_sb[:, 2:4, :, 0:TT], hhi[:, 2:4, :, 0:TT])
            nc.sync.dma_start(wlo_sb[:, 2], wlo[:, 2])
            nc.sync.dma_start(hlo_sb[:, :, :, 0:TT], hlo[:, :, :, 0:TT])
            nc.sync.dma_start(whi_sb[:, 0:2], whi[:, 0:2])
            nc.sync.dma_start(wlo_sb[:, 0:2], wlo[:, 0:2])
            for tt in range(NTT):
                tts_ = bass.ts(tt, TT)
                if tt > 0:
                    h_slice(tt * TT, (tt + 1) * TT)
                nc.sync.dma_start(cos_sb[0:64, tts_], cosd[:, tts_])
                nc.sync.dma_start(sin_sb[0:64, tts_], sind[:, tts_])
                # Pool duplicates the tables onto partitions 64:128
                nc.gpsimd.tensor_copy(cos_sb[64:128, tts_],
                                      cos_sb[0:64, tts_])
                nc.gpsimd.tensor_copy(sin_sb[64:128, tts_],
                                      sin_sb[0:64, tts_])
            nc.sync.dma_start(wohi_sb[:], wohi[:])
            nc.sync.dma_start(wolo_sb[:], wolo[:])

            ident = pp.tile([64, 64], BF, tag="ident")
            make_identity(nc, ident[:])
            ident128 = pp.tile([128, 128], BF, tag="ident128")
            make_identity(nc, ident128[:])
            warm = pp.tile([1, 8], F32, tag="warm")
            nc.any.memset(warm[:], 0.0)
            nc.scalar.activation(warm[:], warm[:], AF.Exp)
            # keep the tensor engine busy on junk matmuls while the first
            # h/w DMAs land, so the p-state ramp completes before real work
            wa = pp.tile([128, TT], BF, tag="wa")
            nc.gpsimd.memset(wa[:], 0.5)

            qrot = [pp.tile([128, S], BF, tag=f"qrot{p}", name=f"qrot{p}")
                    for p in range(2)]
            k2 = pp.tile([128, S], BF, tag="k2")
            vT = pp.tile([64, S], BF, tag="vT")
            vaug = pp.tile([128, KC, 65], BF, tag="vaug")
            nc.any.memset(vaug[:], 1.0 / 32.0)
            # anorm = 32*attn: bf16 full + fp8 hi/lo for the DR outproj,
            # packed [p, oc(=pair), t]
            anorm = pp.tile([128, 2, S], BF, tag="anorm")
            ahi = pp.tile([128, 2, S], E4, tag="ahi")
            alo = pp.tile([128, 2, S], E5, tag="alo")

            with (
                tc.tile_pool(name="psP", bufs=1, space="PSUM") as psP,
                tc.tile_pool(name="psS", bufs=3, space="PSUM") as psS,
                tc.tile_pool(name="psA", bufs=1, space="PSUM") as psA,
            ):

                def rope(pbf, dest, rows, tts, tag):
                    """Pool builds the 32-block-swapped copy; DVE runs
                    same-partition bf16 2x-mode multiply/add ops against the
                    compact shared [64, S] tables."""
                    sg = rp.tile([128, TT], BF, tag=f"sg{tag}")
                    for blk in range(rows // 32):
                        src = blk ^ 1
                        nc.gpsimd.tensor_copy(
                            sg[32 * blk: 32 * blk + 32, :],
                            pbf[32 * src: 32 * src + 32, :])
                    t1 = rp.tile([128, TT], BF, tag=f"t1{tag}")
                    rt = rp.tile([128, TT], BF, tag=f"rt{tag}")
                    nc.vector.tensor_tensor(
                        t1[0:rows, :], pbf[0:rows, :], cos_sb[0:rows, tts],
                        MULT)
                    nc.vector.tensor_tensor(
                        rt[0:rows, :], sg[0:rows, :], sin_sb[0:rows, tts],
                        MULT)
                    nc.vector.tensor_tensor(
                        dest, t1[0:rows, :], rt[0:rows, :], ADD)

                def proj(rc, tt, name, slot):
                    """hi/lo DoubleRow projection: 12 accumulating DR matmuls
                    (4 K=256 blocks x {hi*hi, hi*lo, lo*hi}). slot borrows an
                    idle PSUM ring early in the prologue."""
                    tts = bass.ts(tt, TT)
                    pool, tag = slot
                    if tag == "sc":
                        ps = pool.tile([128, 2 * TT], F32, tag="sc",
                                       name=name)[:, 0:TT]
                    else:
                        ps = pool.tile([128, TT], F32, tag=tag, name=name)
                    steps = []
                    for j in range(4):
                        steps.append((whi_sb[:, rc, j], hhi_sb[:, j, :, tts]))
                    for j in range(4):
                        steps.append((whi_sb[:, rc, j], hlo_sb[:, j, :, tts]))
                        steps.append((wlo_sb[:, rc, j], hhi_sb[:, j, :, tts]))
                    for si, (w, x) in enumerate(steps):
                        nc.tensor.matmul(
                            ps[:], w, x,
                            start=(si == 0), stop=(si == len(steps) - 1),
                            perf_mode=DR)
                    return ps, tts

                def proj_kv(tt, slot=(psP, "proj"), act_copy=False):
                    ps, tts = proj(2, tt, f"projkv_{tt}", slot)
                    kbf = pbfp.tile([128, TT], BF, tag="pbf", name=f"kbf{tt}")
                    if act_copy:
                        nc.scalar.copy(kbf[0:64, :], ps[0:64, :])
                        nc.scalar.copy(vT[:, tts], ps[64:128, :])
                        nc.vector.tensor_scalar_mul(vT[:, tts], vT[:, tts],
                                                    1.0 / 16.0)
                    else:
                        nc.vector.tensor_copy(kbf[0:64, :], ps[0:64, :])
                        # v = ps/16
                        nc.vector.tensor_scalar_mul(vT[:, tts], ps[64:128, :],
                                                    1.0 / 16.0)
                    rope(kbf, k2[0:64, tts], 64, tts, "k")
                    nc.gpsimd.tensor_copy(k2[64:128, tts], k2[0:64, tts])

                def transp(tt, slot=(psP, "proj")):
                    pool, tag = slot
                    pt = pool.tile([128, 4, 64], BF, tag=tag,
                                   name=f"vt{tt}")
                    for ci in range(4):
                        c = 4 * tt + ci
                        nc.tensor.transpose(pt[:, ci, :],
                                            vT[:, bass.ts(c, 128)], ident[:])
                    nc.vector.tensor_copy(vaug[:, 4 * tt: 4 * tt + 4, 0:64],
                                          pt[:])

                def proj_q(rc, tt, slot=(psP, "proj"), act_copy=False):
                    ps, tts = proj(rc, tt, f"projq{rc}_{tt}", slot)
                    pbf = pbfp.tile([128, TT], BF, tag="pbf",
                                    name=f"qbf{rc}_{tt}")
                    nc.vector.tensor_copy(pbf[:], ps[:])
                    rope(pbf, qrot[rc][:, tts], 128, tts, "q")

                def outproj_tile(tch, ht, last=False):
                    tcs = bass.ts(tch, 128)
                    hts = bass.ts(ht, TT)
                    po = psP.tile([128, TT], F32, tag="proj",
                                  name=f"po{tch}_{ht}")
                    terms = [(ahi[:, :, tcs], wohi_sb[:, :, hts]),
                             (ahi[:, :, tcs], wolo_sb[:, :, hts]),
                             (alo[:, :, tcs], wohi_sb[:, :, hts])]
                    for si, (a, w) in enumerate(terms):
                        nc.tensor.matmul(
                            po[:], a, w,
                            start=(si == 0), stop=(si == len(terms) - 1),
                            perf_mode=DR)
                    ob = op_.tile([128, TT], BF, tag="ob")
                    if (tch + ht) % 2 == 0:
                        nc.scalar.copy(ob[:], po[:])
                    else:
                        nc.vector.tensor_copy(ob[:], po[:])
                    nc.sync.dma_start(out[tcs, hts], ob[:])

                # ---- filler queue
                filler = collections.deque()

                def pump(n=1):
                    for _ in range(n):
                        if not filler:
                            return
                        filler.popleft()()

                def attention_head(pair, h2, qt, offload=True,
                                   offload2=False, last_head=False):
                    qts = bass.ts(qt, TT)
                    qrows = slice(64 * h2, 64 * h2 + 64)
                    pacc = psA.tile([128, 4, 65], F32, tag="att",
                                    name=f"att{pair}_{h2}_{qt}")
                    pending = collections.deque()

                    def drain_pending(keep):
                        # transposed attnV: the ex chunk is the STATIONARY
                        # operand and V+ones the moving one -> out free is 65
                        # (attn^T per 128-token chunk, denominator in col 64)
                        while len(pending) > keep:
                            pex, pcp = pending.popleft()
                            for j in range(2):
                                c = 2 * pcp + j
                                for qc in range(4):
                                    # the 4 qc accumulation groups share one
                                    # PSUM zero region: start/stop only once
                                    nc.tensor.matmul(
                                        pacc[:, qc, :],
                                        pex[:, 512 * j + 128 * qc:
                                            512 * j + 128 * qc + 128],
                                        vaug[:, c, :],
                                        start=(c == 0 and qc == 0),
                                        stop=(c == KC - 1 and qc == 3))

                    for cp in range(NCP):
                        sc = psS.tile([128, 2 * TT], F32, tag="sc",
                                      name=f"sc{pair}_{h2}_{qt}_{cp}")
                        for j in range(2):
                            c = 2 * cp + j
                            nc.tensor.matmul(
                                sc[:, bass.ts(j, TT)],
                                k2[qrows, bass.ts(c, 128)],
                                qrot[pair][qrows, qts],
                                start=True, stop=True,
                                tile_position=(64 * h2, 0))
                        if (cp == 3 and offload) or (cp in (1, 6) and offload2):
                            # offload this tile's exp to DVE (Schraudolph);
                            # the ~2% approx error on 1/8 of the keys is
                            # within budget and relieves the pacing engine
                            exi = ep.tile([128, 2 * TT], I16, tag="exps")
                            nc.vector.tensor_scalar(exi[:], sc[:],
                                                    SCH_A, SCH_B, MULT, ADD)
                            ex = exi[:].bitcast(BF)
                        else:
                            ext = ep.tile([128, 2 * TT], BF, tag="exp")
                            nc.scalar.activation(ext[:], sc[:], AF.Exp,
                                                 scale=SCALE)
                            ex = ext[:]
                        pending.append((ex, cp))
                        # attnV trails scores by two cpairs so the exp
                        # semaphore has always fired by the time the PE
                        # reaches the accumulation matmuls
                        drain_pending(3)
                        yield
                    drain_pending(0)
                    # attn^T normalize: the denominator is per-PARTITION ->
                    # Pool normalize_recip does the whole softmax divide;
                    # PE transposes back to [o, t] for the outproj
                    att = np_.tile([128, 4, 65], F32, tag="att_sb")
                    nc.vector.tensor_copy(att[:], pacc[:])
                    nrm = np_.tile([128, 4, 64], BF, tag="nrm")
                    for qc in range(4):
                        nc.gpsimd.normalize_recip(
                            nrm[:, qc, :], att[:, qc, 0:64],
                            att[:, qc, 64:65])
                    pt = psP.tile([64, 4, 128], BF, tag="proj",
                                  name=f"atp{pair}_{h2}_{qt}")
                    for qc in range(4):
                        nc.tensor.transpose(pt[:, qc, :], nrm[:, qc, :],
                                            ident128[:])
                    nc.vector.tensor_copy(anorm[qrows, pair, qts], pt[:])
                    # fp8 hi/lo for the DR outproj (Pool; DVE for the last
                    # head to shorten the tail chain)
                    eng = nc.vector if last_head else nc.gpsimd
                    eng.tensor_copy(ahi[qrows, pair, qts],
                                    anorm[qrows, pair, qts])
                    eng.tensor_tensor(
                        alo[qrows, pair, qts],
                        anorm[qrows, pair, qts],
                        ahi[qrows, pair, qts],
                        SUB)

                # ---- master schedule: kv0/q00 up front with Activation-
                # assisted copies (exp idle), V-transpose 0 borrows the psA
                # ring; later h tiles are DMA-gated so they pump as filler.
                def warm_mms(n, label):
                    for wi in range(n):
                        wps = psA.tile([128, TT], F32, tag="att",
                                       name=f"warm{label}_{wi}")
                        nc.tensor.matmul(wps[:, 0:256], wa[:, 0:128],
                                         wa[:, 0:256], start=True, stop=True)

                warm_mms(16, "a")
                proj_kv(0, slot=(psP, "proj"), act_copy=True)
                proj_q(0, 0, slot=(psP, "proj"), act_copy=True)
                transp(0, slot=(psA, "att"))
                proj_kv(1)
                filler.append(lambda: proj_kv(2))
                filler.append(lambda: transp(1))
                filler.append(lambda: proj_q(1, 0, slot=(psA, "att")))
                filler.append(lambda: proj_kv(3))
                filler.append(lambda: transp(2))
                filler.append(lambda: transp(3))
                for tt in range(1, NTT):
                    for rc in range(2):
                        filler.append(
                            lambda rc=rc, tt=tt: proj_q(rc, tt))

                heads = [(pair, h2, qt)
                         for qt in range(NTT)
                         for pair in range(2)
                         for h2 in range(2)]

                def head_done(i):
                    if i % 4 == 3:
                        qt = heads[i][2]
                        last = i == len(heads) - 1
                        for tch in range(4 * qt, 4 * qt + 4):
                            for ht in range(2):
                                filler.append(
                                    lambda tch=tch, ht=ht, last=last:
                                    outproj_tile(tch, ht, last=last))

                nxt = 0

                def start_next():
                    nonlocal nxt
                    if nxt >= len(heads):
                        return None
                    g = attention_head(*heads[nxt], offload=(nxt >= 1),
                                       offload2=(nxt >= 1),
                                       last_head=(nxt == len(heads) - 1))
                    nxt += 1
                    return (nxt - 1, g)

                nproj_fill = len(filler)
                slots = [start_next(), None]
                stagger = 14
                step = 0
                while any(slots):
                    for si in range(2):
                        if slots[si] is None:
                            continue
                        i, g = slots[si]
                        try:
                            next(g)
                            # projection fillers drain at double rate (their
                            # PSUM->rope chains must stay ahead); outproj
                            # fillers at half rate so they cover the whole
                            # q-tile's rounds instead of bunching
                            if step < nproj_fill:
                                pump(2)
                            elif step % 2 == 0:
                                pump(1)
                            step += 1
                            if stagger is not None:
                                stagger -= 1
                                if stagger == 0:
                                    slots[1] = start_next()
                                    stagger = None
                        except StopIteration:
                            head_done(i)
                            slots[si] = start_next()
                # bridge the last norm chain with junk matmuls so the
                # final outproj tiles run at full PE clock
                filler.appendleft(lambda: warm_mms(14, "t"))
                while filler:
                    pump(1)
    nc.finalize()
    return nc


def _get_nc():
    global _nc_cache
    if _nc_cache is None:
        _nc_cache = _build_bass()
    return _nc_cache


def _hilo(x):
    hi = x.astype(_E4)
    lo = (x - hi.astype(np.float32)).astype(_E5)
    return hi, lo


def _shard_inputs(hidden_states, cos, sin, w_qkv, w_o):
    """Build per-core input maps. Core c = (b = c // 4, g = c % 4)."""
    cosT = cos.T.astype(np.float32)                                # [64, S]
    sinT = sin.T.astype(np.float32)
    sinmod = np.concatenate([-sinT[0:32], sinT[32:64]], axis=0)
    cosc = np.ascontiguousarray(cosT / 16.0).astype(_BF16)
    sinc = np.ascontiguousarray(sinmod / 16.0).astype(_BF16)

    # h packed [p, j, i, t]: feature 256j + 128i + p
    hsplit = []
    for b in range(B):
        ht = hidden_states[b].T.astype(np.float32)                 # [1024, S]
        hp = np.ascontiguousarray(
            ht.reshape(4, 2, 128, S).transpose(2, 0, 1, 3))        # [128,4,2,S]
        hsplit.append(_hilo(hp))
    in_maps = []
    for c in range(NCORES):
        b, g = divmod(c, 4)
        q_rows = w_qkv[256 * g: 256 * g + 256]
        k_rows = w_qkv[1024 + 64 * g: 1024 + 64 * g + 64]
        v_rows = w_qkv[1280 + 64 * g: 1280 + 64 * g + 64]
        wqk = np.concatenate([q_rows, k_rows, v_rows], axis=0)     # [384, 1024]
        # x16 into fp8 range; [p, rc, j, i, m] with h = 256j+128i+p
        wqkT = (wqk.T * 16.0).astype(np.float32)                   # [1024, 384]
        wpk = np.ascontiguousarray(
            wqkT.reshape(4, 2, 128, 3, 128).transpose(2, 3, 0, 1, 4))
        whi_a, wlo_a = _hilo(wpk)
        woTf = (w_o[:, 256 * g: 256 * g + 256].T * 16.0).astype(np.float32)
        wo_pk = np.ascontiguousarray(
            woTf.reshape(2, 128, HID).transpose(1, 0, 2))          # [128,2,HID]
        wohi_a, wolo_a = _hilo(wo_pk)
        in_maps.append(
            {
                "hhi": hsplit[b][0],
                "hlo": hsplit[b][1],
                "whi": whi_a,
                "wlo": wlo_a,
                "wohi": wohi_a,
                "wolo": wolo_a,
                "cosd": cosc,
                "sind": sinc,
            }
        )
    return in_maps


def _run(inputs, **spmd_kwargs):
    from concourse.bass_utils import run_bass_kernel_spmd

    nc = _get_nc()
    in_maps = _shard_inputs(**inputs)
    res = run_bass_kernel_spmd(
        nc, in_maps, core_ids=list(range(NCORES)), **spmd_kwargs
    )
    outs = []
    for b in range(B):
        acc = res.results[4 * b]["out"].astype(np.float32)
        for g in range(1, 4):
            acc = acc + res.results[4 * b + g]["out"].astype(np.float32)
        outs.append(acc * OUT_SCALE)
    return np.stack(outs, axis=0), res


def kernel(**inputs):
    out, _ = _run(inputs)
    return out



# revision 19
# speedup vs baseline: 1.0275x; 1.0275x over previous
"""GQA attention layer (QKV proj + RoPE + softmax attention + out proj) on 8
Trainium2 NeuronCores.

Sharding: core c = (batch b = c//4, head-group g = c%4): 4 q heads + 1 kv
head per core, w_o row-parallel partial output in bf16; the host upcasts,
sums the 4 partials per batch and divides by the fp8 scaling factor (512).

Design highlights vs the original baseline (234 us -> 164 us):
- fp8 hi/lo DoubleRow matmuls for both projections: operands split into
  e4m3 hi + e5m2 lo (host-side for h/w_qkv/w_o, Pool-side for the attention
  output); each K=256 block runs as 3 DR matmuls (hi*hi + hi*lo + lo*hi) at
  0.75x the bf16 PE cost with ~bf16 accuracy. scores/attnV stay bf16
  (single-fp8 would blow the 2e-2 error budget).
- SCALE folded into the exp activation so q and k share one compact rope
  table; exp runs on Activation in [128, 1024] tiles; 1-2 tiles per head
  (~18% of keys) offload to DVE via a Schraudolph bitcast exp
  (int16(A*s + B) reinterpreted as bf16), weighted toward rounds where the
  PE has no filler work.
- softmax normalize: pacc copied off PSUM immediately (the psA bank recycles
  in one op), reciprocal on a partition-0 staged denominator (PSUM-sourced
  or partition-offset reciprocal inputs return garbage on HW), Pool
  partition_broadcast + Pool multiply replace the baseline's fp32 PE
  broadcast matmuls (everything SBUF-side is Pool-legal).
- schedule: two staggered attention-head generators with a filler queue
  (projections at 2x pump rate, V transposes, outproj tiles at 1/4 rate to
  cover whole q-tiles); junk warm-up matmuls bridge the initial DMA wait and
  the final norm chain so the PE p-state never drops mid-kernel; DMA order
  tuned so each k/v tile lands just before the attention wavefront needs it.
"""

import collections

import numpy as np
import ml_dtypes

B, S, HID = 2, 2048, 1024
NH, NKV, D = 16, 4, 64
SCALE = float(D ** -0.5)
NCORES = 8
TT = 512          # token tile
NTT = S // TT     # 4
KC = S // 128     # 16 key chunks
NCP = KC // 2     # 8 chunk pairs
OUT_SCALE = 1.0 / 512.0

_BF16 = ml_dtypes.bfloat16
_E4 = ml_dtypes.float8_e4m3
_E5 = ml_dtypes.float8_e5m2

_nc_cache = None


def _build_bass():
    import concourse.bass as bass
    import concourse.mybir as mybir
    import concourse.tile as tile
    from concourse import bacc
    from concourse.masks import make_identity

    BF = mybir.dt.bfloat16
    F32 = mybir.dt.float32
    I16 = mybir.dt.int16
    E4 = mybir.dt.float8e4
    E5 = mybir.dt.float8e5
    AF = mybir.ActivationFunctionType
    MULT = mybir.AluOpType.mult
    ADD = mybir.AluOpType.add
    SUB = mybir.AluOpType.subtract
    DR = mybir.MatmulPerfMode.DoubleRow
    # Schraudolph exp for offloaded tiles: bitcast(int16(s*A + B)) ~ exp(s)
    SCH_A = 184.66496280558537 * SCALE   # 128/ln2, scores carry 1/SCALE
    SCH_B = 16256.0 - 5.75 + 0.5         # bias center + truncation fix

    nc = bacc.Bacc()
    # h split hi/lo, packed [p, j, i, t]: h feature 256j + 128i + p
    hhi = nc.dram_tensor("hhi", (128, 4, 2, S), E4, kind="ExternalInput")
    hlo = nc.dram_tensor("hlo", (128, 4, 2, S), E5, kind="ExternalInput")
    # wqk split hi/lo, packed [p, rc, j, i, m]
    whi = nc.dram_tensor("whi", (128, 3, 4, 2, 128), E4, kind="ExternalInput")
    wlo = nc.dram_tensor("wlo", (128, 3, 4, 2, 128), E5, kind="ExternalInput")
    wohi = nc.dram_tensor("wohi", (128, 2, HID), E4, kind="ExternalInput")
    wolo = nc.dram_tensor("wolo", (128, 2, HID), E5, kind="ExternalInput")
    # shared q/k rope tables (/16); SCALE is applied by the exp activation
    cosd = nc.dram_tensor("cosd", (64, S), BF, kind="ExternalInput")
    sind = nc.dram_tensor("sind", (64, S), BF, kind="ExternalInput")
    out = nc.dram_tensor("out", (S, HID), BF, kind="ExternalOutput")

    with tile.TileContext(nc) as tc:
        with (
            tc.tile_pool(name="persist", bufs=1) as pp,
            tc.tile_pool(name="pbfp", bufs=3) as pbfp,
            tc.tile_pool(name="rope", bufs=3) as rp,
            tc.tile_pool(name="exps", bufs=6) as ep,
            tc.tile_pool(name="norm", bufs=4) as np_,
            tc.tile_pool(name="outsb", bufs=4) as op_,
        ):
            # ---- persistent SBUF tiles + input loads, kv-first order
            hhi_sb = pp.tile([128, 4, 2, S], E4, tag="hhi_sb")
            hlo_sb = pp.tile([128, 4, 2, S], E5, tag="hlo_sb")
            whi_sb = pp.tile([128, 3, 4, 2, 128], E4, tag="whi_sb")
            wlo_sb = pp.tile([128, 3, 4, 2, 128], E5, tag="wlo_sb")
            wohi_sb = pp.tile([128, 2, HID], E4, tag="wohi_sb")
            wolo_sb = pp.tile([128, 2, HID], E5, tag="wolo_sb")
            # [128, S]: rows 0:64 DMA'd, rows 64:128 duplicated on-device
            cos_sb = pp.tile([128, S], BF, tag="cos_sb")
            sin_sb = pp.tile([128, S], BF, tag="sin_sb")

            def h_slice(t0, t1):
                for hd, hs in ((hhi, hhi_sb), (hlo, hlo_sb)):
                    nc.sync.dma_start(hs[:, :, :, t0:t1], hd[:, :, :, t0:t1])

            def cossin(tt):
                tts_ = bass.ts(tt, TT)
                nc.sync.dma_start(cos_sb[0:64, tts_], cosd[:, tts_])
                nc.sync.dma_start(sin_sb[0:64, tts_], sind[:, tts_])
                # Pool duplicates the tables onto partitions 64:128
                nc.gpsimd.tensor_copy(cos_sb[64:128, tts_],
                                      cos_sb[0:64, tts_])
                nc.gpsimd.tensor_copy(sin_sb[64:128, tts_],
                                      sin_sb[0:64, tts_])

            nc.sync.dma_start(whi_sb[:, 2], whi[:, 2])
            nc.sync.dma_start(hhi_sb[:, 0:2, :, 0:TT], hhi[:, 0:2, :, 0:TT])
            nc.sync.dma_start(hhi_sb[:, 2:4, :, 0:TT], hhi[:, 2:4, :, 0:TT])
            nc.sync.dma_start(wlo_sb[:, 2], wlo[:, 2])
            nc.sync.dma_start(hlo_sb[:, :, :, 0:TT], hlo[:, :, :, 0:TT])
            cossin(0)  # rope for kv0/q00 needs the tables right away
            nc.sync.dma_start(whi_sb[:, 0:2], whi[:, 0:2])
            nc.sync.dma_start(wlo_sb[:, 0:2], wlo[:, 0:2])
            for tt in range(1, NTT):
                h_slice(tt * TT, (tt + 1) * TT)
                cossin(tt)
            nc.sync.dma_start(wohi_sb[:], wohi[:])
            nc.sync.dma_start(wolo_sb[:], wolo[:])

            ident = pp.tile([64, 64], BF, tag="ident")
            make_identity(nc, ident[:])
            ident128 = pp.tile([128, 128], BF, tag="ident128")
            make_identity(nc, ident128[:])
            warm = pp.tile([1, 8], F32, tag="warm")
            nc.any.memset(warm[:], 0.0)
            nc.scalar.activation(warm[:], warm[:], AF.Exp)
            # keep the tensor engine busy on junk matmuls while the first
            # h/w DMAs land, so the p-state ramp completes before real work
            wa = pp.tile([128, TT], BF, tag="wa")
            nc.gpsimd.memset(wa[:], 0.5)

            qrot = [pp.tile([128, S], BF, tag=f"qrot{p}", name=f"qrot{p}")
                    for p in range(2)]
            k2 = pp.tile([128, S], BF, tag="k2")
            vT = pp.tile([64, S], BF, tag="vT")
            vaug = pp.tile([128, KC, 65], BF, tag="vaug")
            nc.any.memset(vaug[:], 1.0 / 32.0)
            # anorm = 32*attn: bf16 full + fp8 hi/lo for the DR outproj,
            # packed [p, oc(=pair), t]
            anorm = pp.tile([128, 2, S], BF, tag="anorm")
            ahi = pp.tile([128, 2, S], E4, tag="ahi")
            alo = pp.tile([128, 2, S], E5, tag="alo")

            with (
                tc.tile_pool(name="psP", bufs=1, space="PSUM") as psP,
                tc.tile_pool(name="psS", bufs=3, space="PSUM") as psS,
                tc.tile_pool(name="psA", bufs=1, space="PSUM") as psA,
            ):

                def rope(pbf, dest, rows, tts, tag, eng=None):
                    """Pool builds the 32-block-swapped copy; the multiply/add
                    ops (all-SBUF bf16) default to DVE 2x mode but can run on
                    Pool when DVE is the busier engine."""
                    if eng is None:
                        eng = nc.vector
                    sg = rp.tile([128, TT], BF, tag=f"sg{tag}")
                    for blk in range(rows // 32):
                        src = blk ^ 1
                        nc.gpsimd.tensor_copy(
                            sg[32 * blk: 32 * blk + 32, :],
                            pbf[32 * src: 32 * src + 32, :])
                    t1 = rp.tile([128, TT], BF, tag=f"t1{tag}")
                    rt = rp.tile([128, TT], BF, tag=f"rt{tag}")
                    eng.tensor_tensor(
                        t1[0:rows, :], pbf[0:rows, :], cos_sb[0:rows, tts],
                        MULT)
                    eng.tensor_tensor(
                        rt[0:rows, :], sg[0:rows, :], sin_sb[0:rows, tts],
                        MULT)
                    eng.tensor_tensor(
                        dest, t1[0:rows, :], rt[0:rows, :], ADD)

                def proj(rc, tt, name, slot):
                    """hi/lo DoubleRow projection: 12 accumulating DR matmuls
                    (4 K=256 blocks x {hi*hi, hi*lo, lo*hi}). slot borrows an
                    idle PSUM ring early in the prologue."""
                    tts = bass.ts(tt, TT)
                    pool, tag = slot
                    if tag == "sc":
                        ps = pool.tile([128, 2 * TT], F32, tag="sc",
                                       name=name)[:, 0:TT]
                    else:
                        ps = pool.tile([128, TT], F32, tag=tag, name=name)
                    steps = []
                    for j in range(4):
                        steps.append((whi_sb[:, rc, j], hhi_sb[:, j, :, tts]))
                    for j in range(4):
                        steps.append((whi_sb[:, rc, j], hlo_sb[:, j, :, tts]))
                        steps.append((wlo_sb[:, rc, j], hhi_sb[:, j, :, tts]))
                    for si, (w, x) in enumerate(steps):
                        nc.tensor.matmul(
                            ps[:], w, x,
                            start=(si == 0), stop=(si == len(steps) - 1),
                            perf_mode=DR)
                    return ps, tts

                def proj_kv(tt, slot=(psP, "proj"), act_copy=False):
                    ps, tts = proj(2, tt, f"projkv_{tt}", slot)
                    kbf = pbfp.tile([128, TT], BF, tag="pbf", name=f"kbf{tt}")
                    if act_copy:
                        nc.scalar.copy(kbf[0:64, :], ps[0:64, :])
                        nc.scalar.copy(vT[:, tts], ps[64:128, :])
                        nc.vector.tensor_scalar_mul(vT[:, tts], vT[:, tts],
                                                    1.0 / 16.0)
                    else:
                        nc.vector.tensor_copy(kbf[0:64, :], ps[0:64, :])
                        # v = ps/16
                        nc.vector.tensor_scalar_mul(vT[:, tts], ps[64:128, :],
                                                    1.0 / 16.0)
                    rope(kbf, k2[0:64, tts], 64, tts, "k")
                    nc.gpsimd.tensor_copy(k2[64:128, tts], k2[0:64, tts])

                def transp(tt, slot=(psP, "proj")):
                    pool, tag = slot
                    pt = pool.tile([128, 4, 64], BF, tag=tag,
                                   name=f"vt{tt}")
                    for ci in range(4):
                        c = 4 * tt + ci
                        nc.tensor.transpose(pt[:, ci, :],
                                            vT[:, bass.ts(c, 128)], ident[:])
                    nc.vector.tensor_copy(vaug[:, 4 * tt: 4 * tt + 4, 0:64],
                                          pt[:])

                def proj_q(rc, tt, slot=(psP, "proj"), act_copy=False):
                    ps, tts = proj(rc, tt, f"projq{rc}_{tt}", slot)
                    pbf = pbfp.tile([128, TT], BF, tag="pbf",
                                    name=f"qbf{rc}_{tt}")
                    nc.vector.tensor_copy(pbf[:], ps[:])
                    # later q-tiles rope on Pool: they are pumped well ahead
                    # of their heads, and it keeps DVE free for PSUM drains
                    reng = nc.gpsimd if tt >= 1 else nc.vector
                    rope(pbf, qrot[rc][:, tts], 128, tts, "q", eng=reng)

                def outproj_tile(tch, ht, last=False, slot=(psP, "proj")):
                    tcs = bass.ts(tch, 128)
                    hts = bass.ts(ht, TT)
                    pool, tag = slot
                    if tag == "sc":
                        po = pool.tile([128, 2 * TT], F32, tag="sc",
                                       name=f"po{tch}_{ht}")[:, 0:TT]
                    else:
                        po = pool.tile([128, TT], F32, tag=tag,
                                       name=f"po{tch}_{ht}")
                    terms = [(ahi[:, :, tcs], wohi_sb[:, :, hts]),
                             (ahi[:, :, tcs], wolo_sb[:, :, hts]),
                             (alo[:, :, tcs], wohi_sb[:, :, hts])]
                    for si, (a, w) in enumerate(terms):
                        nc.tensor.matmul(
                            po[:], a, w,
                            start=(si == 0), stop=(si == len(terms) - 1),
                            perf_mode=DR)
                    ob = op_.tile([128, TT], BF, tag="ob")
                    # PSUM drain: only Act/DVE may read PSUM; DVE is the
                    # cheaper copier so it takes 2 of every 3
                    eng = ob_ctr[0] % 3
                    ob_ctr[0] += 1
                    if eng == 1:
                        nc.scalar.copy(ob[:], po[:])
                    else:
                        nc.vector.tensor_copy(ob[:], po[:])
                    nc.sync.dma_start(out[tcs, hts], ob[:])

                # ---- filler queue
                filler = collections.deque()
                ob_ctr = [0]

                def pump(n=1):
                    for _ in range(n):
                        if not filler:
                            return
                        filler.popleft()()

                def attention_head(pair, h2, qt, exp_map=None,
                                   last_head=False):
                    if exp_map is None:
                        exp_map = {}
                    qts = bass.ts(qt, TT)
                    qrows = slice(64 * h2, 64 * h2 + 64)
                    pacc = psA.tile([128, 4, 65], F32, tag="att",
                                    name=f"att{pair}_{h2}_{qt}")
                    pending = collections.deque()

                    def drain_pending(keep):
                        # transposed attnV: the ex chunk is the STATIONARY
                        # operand and V+ones the moving one -> out free is 65
                        # (attn^T per 128-token chunk, denominator in col 64)
                        while len(pending) > keep:
                            pex, pcp = pending.popleft()
                            for j in range(2):
                                c = 2 * pcp + j
                                for qc in range(4):
                                    # the 4 qc accumulation groups share one
                                    # PSUM zero region: start/stop only once
                                    nc.tensor.matmul(
                                        pacc[:, qc, :],
                                        pex[:, 512 * j + 128 * qc:
                                            512 * j + 128 * qc + 128],
                                        vaug[:, c, :],
                                        start=(c == 0 and qc == 0),
                                        stop=(c == KC - 1 and qc == 3))

                    for cp in range(NCP):
                        sc = psS.tile([128, 2 * TT], F32, tag="sc",
                                      name=f"sc{pair}_{h2}_{qt}_{cp}")
                        for j in range(2):
                            c = 2 * cp + j
                            nc.tensor.matmul(
                                sc[:, bass.ts(j, TT)],
                                k2[qrows, bass.ts(c, 128)],
                                qrot[pair][qrows, qts],
                                start=True, stop=True,
                                tile_position=(64 * h2, 0))
                        if exp_map.get(cp) == "dve":
                            # offload this tile's exp to DVE (Schraudolph);
                            # ~2% approx error on those keys is within budget
                            # and relieves the pacing Activation engine.
                            # (Pool cannot read PSUM, so only DVE can assist.)
                            exi = ep.tile([128, 2 * TT], I16, tag="exps")
                            nc.vector.tensor_scalar(exi[:], sc[:],
                                                    SCH_A, SCH_B, MULT, ADD)
                            ex = exi[:].bitcast(BF)
                        else:
                            ext = ep.tile([128, 2 * TT], BF, tag="exp")
                            nc.scalar.activation(ext[:], sc[:], AF.Exp,
                                                 scale=SCALE)
                            ex = ext[:]
                        pending.append((ex, cp))
                        # attnV trails scores by two cpairs so the exp
                        # semaphore has always fired by the time the PE
                        # reaches the accumulation matmuls
                        drain_pending(3)
                        yield
                    drain_pending(0)
                    # attn^T normalize: the denominator is per-PARTITION ->
                    # Pool normalize_recip does the whole softmax divide;
                    # PE transposes back to [o, t] for the outproj
                    att = np_.tile([128, 4, 65], F32, tag="att_sb")
                    nc.vector.tensor_copy(att[:], pacc[:])
                    nrm = np_.tile([128, 4, 64], BF, tag="nrm")
                    for qc in range(4):
                        nc.gpsimd.normalize_recip(
                            nrm[:, qc, :], att[:, qc, 0:64],
                            att[:, qc, 64:65])
                    pt = psP.tile([64, 4, 128], BF, tag="proj",
                                  name=f"atp{pair}_{h2}_{qt}")
                    for qc in range(4):
                        nc.tensor.transpose(pt[:, qc, :], nrm[:, qc, :],
                                            ident128[:])
                    nc.vector.tensor_copy(anorm[qrows, pair, qts], pt[:])
                    # fp8 hi/lo for the DR outproj on Pool (all-SBUF; Pool is
                    # the idle engine at the tail)
                    nc.gpsimd.tensor_copy(ahi[qrows, pair, qts],
                                          anorm[qrows, pair, qts])
                    nc.gpsimd.tensor_tensor(
                        alo[qrows, pair, qts],
                        anorm[qrows, pair, qts],
                        ahi[qrows, pair, qts],
                        SUB)

                # ---- master schedule: kv0/q00 up front with Activation-
                # assisted copies (exp idle), V-transpose 0 borrows the psA
                # ring; later h tiles are DMA-gated so they pump as filler.
                def warm_mms(n, label):
                    for wi in range(n):
                        wps = psA.tile([128, TT], F32, tag="att",
                                       name=f"warm{label}_{wi}")
                        nc.tensor.matmul(wps[:, 0:256], wa[:, 0:128],
                                         wa[:, 0:256], start=True, stop=True)

                warm_mms(16, "a")
                # prologue projections spread across the still-idle psS banks
                # so consecutive projections never serialize on one PSUM bank
                proj_kv(0, slot=(psP, "proj"), act_copy=True)
                proj_q(0, 0, slot=(psS, "sc"), act_copy=True)
                transp(0, slot=(psA, "att"))
                proj_kv(1, slot=(psS, "sc"))
                filler.append(lambda: proj_kv(2, slot=(psS, "sc")))
                filler.append(lambda: transp(1))
                filler.append(lambda: proj_q(1, 0, slot=(psA, "att")))
                filler.append(lambda: proj_kv(3))
                filler.append(lambda: transp(2))
                filler.append(lambda: transp(3))
                for tt in range(1, NTT):
                    for rc in range(2):
                        filler.append(
                            lambda rc=rc, tt=tt: proj_q(rc, tt))

                heads = [(pair, h2, qt)
                         for qt in range(NTT)
                         for pair in range(2)
                         for h2 in range(2)]

                # tail (qt3) tiles alternate psP/psS PSUM slots so consecutive
                # po matmuls never wait on the previous tile's ob drain
                tail_ctr = [0]

                def tail_slot():
                    s = ((psP, "proj") if tail_ctr[0] % 2 == 0
                         else (psS, "sc"))
                    tail_ctr[0] += 1
                    return s

                def head_done(i):
                    if i % 4 == 3:
                        qt = heads[i][2]
                        last = i == len(heads) - 1
                        for tch in range(4 * qt, 4 * qt + 4):
                            for ht in range(2):
                                if qt == 3:
                                    filler.append(
                                        lambda tch=tch, ht=ht, last=last:
                                        outproj_tile(tch, ht, last=last,
                                                     slot=tail_slot()))
                                else:
                                    filler.append(
                                        lambda tch=tch, ht=ht, last=last:
                                        outproj_tile(tch, ht, last=last))

                nxt = 0
                # per-head exp engine map: Act is exact, DVE runs the
                # Schraudolph approximation. head 0 stays mostly exact (its
                # exps overlap the idle prologue Act).
                def exp_map_for(i):
                    if i == 0:
                        return {6: "dve"}
                    return {1: "dve", 3: "dve", 6: "dve"}

                def start_next():
                    nonlocal nxt
                    if nxt >= len(heads):
                        return None
                    g = attention_head(*heads[nxt],
                                       exp_map=exp_map_for(nxt),
                                       last_head=(nxt == len(heads) - 1))
                    nxt += 1
                    return (nxt - 1, g)

                nproj_fill = len(filler)
                slots = [start_next(), None]
                stagger = 14
                step = 0
                while any(slots):
                    for si in range(2):
                        if slots[si] is None:
                            continue
                        i, g = slots[si]
                        try:
                            next(g)
                            # projection fillers drain at double rate (their
                            # PSUM->rope chains must stay ahead); outproj
                            # fillers at half rate so they cover the whole
                            # q-tile's rounds instead of bunching
                            if step < nproj_fill:
                                pump(2)
                            elif step % 2 == 0:
                                pump(1)
                            step += 1
                            if stagger is not None:
                                stagger -= 1
                                if stagger == 0:
                                    slots[1] = start_next()
                                    stagger = None
                        except StopIteration:
                            head_done(i)
                            slots[si] = start_next()
                # bridge the last norm chain with junk matmuls so the
                # final outproj tiles run at full PE clock
                filler.appendleft(lambda: warm_mms(10, "t"))
                while filler:
                    pump(1)
    nc.finalize()
    return nc


def _get_nc():
    global _nc_cache
    if _nc_cache is None:
        _nc_cache = _build_bass()
    return _nc_cache


def _hilo(x):
    hi = x.astype(_E4)
    lo = (x - hi.astype(np.float32)).astype(_E5)
    return hi, lo


def _shard_inputs(hidden_states, cos, sin, w_qkv, w_o):
    """Build per-core input maps. Core c = (b = c // 4, g = c % 4)."""
    cosT = cos.T.astype(np.float32)                                # [64, S]
    sinT = sin.T.astype(np.float32)
    sinmod = np.concatenate([-sinT[0:32], sinT[32:64]], axis=0)
    cosc = np.ascontiguousarray(cosT / 16.0).astype(_BF16)
    sinc = np.ascontiguousarray(sinmod / 16.0).astype(_BF16)

    # h packed [p, j, i, t]: feature 256j + 128i + p
    hsplit = []
    for b in range(B):
        ht = hidden_states[b].T.astype(np.float32)                 # [1024, S]
        hp = np.ascontiguousarray(
            ht.reshape(4, 2, 128, S).transpose(2, 0, 1, 3))        # [128,4,2,S]
        hsplit.append(_hilo(hp))
    in_maps = []
    for c in range(NCORES):
        b, g = divmod(c, 4)
        q_rows = w_qkv[256 * g: 256 * g + 256]
        k_rows = w_qkv[1024 + 64 * g: 1024 + 64 * g + 64]
        v_rows = w_qkv[1280 + 64 * g: 1280 + 64 * g + 64]
        wqk = np.concatenate([q_rows, k_rows, v_rows], axis=0)     # [384, 1024]
        # x16 into fp8 range; [p, rc, j, i, m] with h = 256j+128i+p
        wqkT = (wqk.T * 16.0).astype(np.float32)                   # [1024, 384]
        wpk = np.ascontiguousarray(
            wqkT.reshape(4, 2, 128, 3, 128).transpose(2, 3, 0, 1, 4))
        whi_a, wlo_a = _hilo(wpk)
        woTf = (w_o[:, 256 * g: 256 * g + 256].T * 16.0).astype(np.float32)
        wo_pk = np.ascontiguousarray(
            woTf.reshape(2, 128, HID).transpose(1, 0, 2))          # [128,2,HID]
        wohi_a, wolo_a = _hilo(wo_pk)
        in_maps.append(
            {
                "hhi": hsplit[b][0],
                "hlo": hsplit[b][1],
                "whi": whi_a,
                "wlo": wlo_a,
                "wohi": wohi_a,
                "wolo": wolo_a,
                "cosd": cosc,
                "sind": sinc,
            }
        )
    return in_maps


def _run(inputs, **spmd_kwargs):
    from concourse.bass_utils import run_bass_kernel_spmd

    nc = _get_nc()
    in_maps = _shard_inputs(**inputs)
    res = run_bass_kernel_spmd(
        nc, in_maps, core_ids=list(range(NCORES)), **spmd_kwargs
    )
    outs = []
    for b in range(B):
        acc = res.results[4 * b]["out"].astype(np.float32)
        for g in range(1, 4):
            acc = acc + res.results[4 * b + g]["out"].astype(np.float32)
        outs.append(acc * OUT_SCALE)
    return np.stack(outs, axis=0), res


def kernel(**inputs):
    out, _ = _run(inputs)
    return out



# revision 37
# speedup vs baseline: 1.0615x; 1.0331x over previous
"""GQA attention layer (QKV proj + RoPE + softmax attention + out proj) on 8
Trainium2 NeuronCores.

Sharding: core c = (batch b = c//4, head-group g = c%4): 4 q heads + 1 kv
head per core, w_o row-parallel partial output in bf16; the host upcasts,
sums the 4 partials per batch and divides by the fp8 scaling factor (512).

Design highlights vs the original baseline (234 us -> 164 us):
- fp8 hi/lo DoubleRow matmuls for both projections: operands split into
  e4m3 hi + e5m2 lo (host-side for h/w_qkv/w_o, Pool-side for the attention
  output); each K=256 block runs as 3 DR matmuls (hi*hi + hi*lo + lo*hi) at
  0.75x the bf16 PE cost with ~bf16 accuracy. scores/attnV stay bf16
  (single-fp8 would blow the 2e-2 error budget).
- SCALE folded into the exp activation so q and k share one compact rope
  table; exp runs on Activation in [128, 1024] tiles; 1-2 tiles per head
  (~18% of keys) offload to DVE via a Schraudolph bitcast exp
  (int16(A*s + B) reinterpreted as bf16), weighted toward rounds where the
  PE has no filler work.
- softmax normalize: pacc copied off PSUM immediately (the psA bank recycles
  in one op), reciprocal on a partition-0 staged denominator (PSUM-sourced
  or partition-offset reciprocal inputs return garbage on HW), Pool
  partition_broadcast + Pool multiply replace the baseline's fp32 PE
  broadcast matmuls (everything SBUF-side is Pool-legal).
- schedule: two staggered attention-head generators with a filler queue
  (projections at 2x pump rate, V transposes, outproj tiles at 1/4 rate to
  cover whole q-tiles); junk warm-up matmuls bridge the initial DMA wait and
  the final norm chain so the PE p-state never drops mid-kernel; DMA order
  tuned so each k/v tile lands just before the attention wavefront needs it.
"""

import collections

import numpy as np
import ml_dtypes

B, S, HID = 2, 2048, 1024
NH, NKV, D = 16, 4, 64
SCALE = float(D ** -0.5)
NCORES = 8
TT = 512          # token tile
NTT = S // TT     # 4
KC = S // 128     # 16 key chunks
NCP = KC // 2     # 8 chunk pairs
OUT_SCALE = 1.0 / 512.0

_BF16 = ml_dtypes.bfloat16
_E4 = ml_dtypes.float8_e4m3
_E5 = ml_dtypes.float8_e5m2

_nc_cache = None

# schedule knobs (tuned by simulation sweep; defaults are the shipped config)
CFG = {
    "transp_eng": "act",   # engine for the vaug drain after V transposes
    "q10_slot": "psA",     # PSUM slot for the prologue proj_q(1, 0)
    "dma_hlo_early": True,  # issue hlo tile0 before wlo2
    "warm_n": 22,           # prologue warm-up matmuls
    "ob_pat": "alt2",       # outproj PSUM-drain engine pattern
    "stagger": 12,          # rounds before the second head slot starts
    "exp_pat": "a",         # which chunk-pairs Schraudolph on DVE per head
    "pump_div": 2,          # outproj filler pump cadence (1 per N rounds)
    "keep_n": 6,            # attnV trails scores by keep_n chunk-pairs
}


def _build_bass():
    import concourse.bass as bass
    import concourse.mybir as mybir
    import concourse.tile as tile
    from concourse import bacc
    from concourse.masks import make_identity

    BF = mybir.dt.bfloat16
    F32 = mybir.dt.float32
    I16 = mybir.dt.int16
    E4 = mybir.dt.float8e4
    E5 = mybir.dt.float8e5
    AF = mybir.ActivationFunctionType
    MULT = mybir.AluOpType.mult
    ADD = mybir.AluOpType.add
    SUB = mybir.AluOpType.subtract
    DR = mybir.MatmulPerfMode.DoubleRow
    # Schraudolph exp for offloaded tiles: bitcast(int16(s*A + B)) ~ exp(s)
    SCH_A = 184.66496280558537 * SCALE   # 128/ln2, scores carry 1/SCALE
    SCH_B = 16256.0 - 5.75 + 0.5         # bias center + truncation fix

    nc = bacc.Bacc()
    # h split hi/lo, packed [p, j, i, t]: h feature 256j + 128i + p
    hhi = nc.dram_tensor("hhi", (128, 4, 2, S), E4, kind="ExternalInput")
    hlo = nc.dram_tensor("hlo", (128, 4, 2, S), E5, kind="ExternalInput")
    # wqk split hi/lo, packed [p, rc, j, i, m]
    whi = nc.dram_tensor("whi", (128, 3, 4, 2, 128), E4, kind="ExternalInput")
    wlo = nc.dram_tensor("wlo", (128, 3, 4, 2, 128), E5, kind="ExternalInput")
    wohi = nc.dram_tensor("wohi", (128, 2, HID), E4, kind="ExternalInput")
    wolo = nc.dram_tensor("wolo", (128, 2, HID), E5, kind="ExternalInput")
    # shared q/k rope tables (/16); SCALE is applied by the exp activation
    cosd = nc.dram_tensor("cosd", (64, S), BF, kind="ExternalInput")
    sind = nc.dram_tensor("sind", (64, S), BF, kind="ExternalInput")
    out = nc.dram_tensor("out", (S, HID), BF, kind="ExternalOutput")

    with tile.TileContext(nc) as tc:
        with (
            tc.tile_pool(name="persist", bufs=1) as pp,
            tc.tile_pool(name="pbfp", bufs=3) as pbfp,
            tc.tile_pool(name="rope", bufs=3) as rp,
            tc.tile_pool(name="exps", bufs=6) as ep,
            tc.tile_pool(name="norm", bufs=4) as np_,
            tc.tile_pool(name="outsb", bufs=4) as op_,
        ):
            # ---- persistent SBUF tiles + input loads, kv-first order
            hhi_sb = pp.tile([128, 4, 2, S], E4, tag="hhi_sb")
            hlo_sb = pp.tile([128, 4, 2, S], E5, tag="hlo_sb")
            whi_sb = pp.tile([128, 3, 4, 2, 128], E4, tag="whi_sb")
            wlo_sb = pp.tile([128, 3, 4, 2, 128], E5, tag="wlo_sb")
            wohi_sb = pp.tile([128, 2, HID], E4, tag="wohi_sb")
            wolo_sb = pp.tile([128, 2, HID], E5, tag="wolo_sb")
            # [128, S]: rows 0:64 DMA'd, rows 64:128 duplicated on-device
            cos_sb = pp.tile([128, S], BF, tag="cos_sb")
            sin_sb = pp.tile([128, S], BF, tag="sin_sb")

            def h_slice(t0, t1):
                for hd, hs in ((hhi, hhi_sb), (hlo, hlo_sb)):
                    nc.sync.dma_start(hs[:, :, :, t0:t1], hd[:, :, :, t0:t1])

            def cossin(tt):
                tts_ = bass.ts(tt, TT)
                nc.sync.dma_start(cos_sb[0:64, tts_], cosd[:, tts_])
                nc.sync.dma_start(sin_sb[0:64, tts_], sind[:, tts_])
                # Pool duplicates the tables onto partitions 64:128
                nc.gpsimd.tensor_copy(cos_sb[64:128, tts_],
                                      cos_sb[0:64, tts_])
                nc.gpsimd.tensor_copy(sin_sb[64:128, tts_],
                                      sin_sb[0:64, tts_])

            nc.sync.dma_start(whi_sb[:, 2], whi[:, 2])
            nc.sync.dma_start(hhi_sb[:, 0:2, :, 0:TT], hhi[:, 0:2, :, 0:TT])
            nc.sync.dma_start(hhi_sb[:, 2:4, :, 0:TT], hhi[:, 2:4, :, 0:TT])
            if CFG["dma_hlo_early"]:
                nc.sync.dma_start(hlo_sb[:, :, :, 0:TT], hlo[:, :, :, 0:TT])
                nc.sync.dma_start(wlo_sb[:, 2], wlo[:, 2])
            else:
                nc.sync.dma_start(wlo_sb[:, 2], wlo[:, 2])
                nc.sync.dma_start(hlo_sb[:, :, :, 0:TT], hlo[:, :, :, 0:TT])
            cossin(0)  # rope for kv0/q00 needs the tables right away
            nc.sync.dma_start(whi_sb[:, 0:2], whi[:, 0:2])
            nc.sync.dma_start(wlo_sb[:, 0:2], wlo[:, 0:2])
            for tt in range(1, NTT):
                h_slice(tt * TT, (tt + 1) * TT)
                cossin(tt)
            nc.sync.dma_start(wohi_sb[:], wohi[:])
            nc.sync.dma_start(wolo_sb[:], wolo[:])

            ident = pp.tile([64, 64], BF, tag="ident")
            make_identity(nc, ident[:])
            ident128 = pp.tile([128, 128], BF, tag="ident128")
            make_identity(nc, ident128[:])
            warm = pp.tile([1, 8], F32, tag="warm")
            nc.any.memset(warm[:], 0.0)
            nc.scalar.activation(warm[:], warm[:], AF.Exp)
            # keep the tensor engine busy on junk matmuls while the first
            # h/w DMAs land, so the p-state ramp completes before real work
            wa = pp.tile([128, TT], BF, tag="wa")
            nc.gpsimd.memset(wa[:], 0.5)

            qrot = [pp.tile([128, S], BF, tag=f"qrot{p}", name=f"qrot{p}")
                    for p in range(2)]
            k2 = pp.tile([128, S], BF, tag="k2")
            vT = pp.tile([64, S], BF, tag="vT")
            vaug = pp.tile([128, KC, 65], BF, tag="vaug")
            nc.any.memset(vaug[:], 1.0 / 32.0)
            # anorm = 32*attn: bf16 full + fp8 hi/lo for the DR outproj,
            # packed [p, oc(=pair), t]
            anorm = pp.tile([128, 2, S], BF, tag="anorm")
            ahi = pp.tile([128, 2, S], E4, tag="ahi")
            alo = pp.tile([128, 2, S], E5, tag="alo")

            with (
                tc.tile_pool(name="psP", bufs=1, space="PSUM") as psP,
                tc.tile_pool(name="psS", bufs=3, space="PSUM") as psS,
                tc.tile_pool(name="psA", bufs=1, space="PSUM") as psA,
            ):

                def rope(pbf, dest, rows, tts, tag, eng=None):
                    """Pool builds the 32-block-swapped copy; the multiply/add
                    ops (all-SBUF bf16) default to DVE 2x mode but can run on
                    Pool when DVE is the busier engine."""
                    if eng is None:
                        eng = nc.vector
                    sg = rp.tile([128, TT], BF, tag=f"sg{tag}")
                    for blk in range(rows // 32):
                        src = blk ^ 1
                        nc.gpsimd.tensor_copy(
                            sg[32 * blk: 32 * blk + 32, :],
                            pbf[32 * src: 32 * src + 32, :])
                    t1 = rp.tile([128, TT], BF, tag=f"t1{tag}")
                    rt = rp.tile([128, TT], BF, tag=f"rt{tag}")
                    eng.tensor_tensor(
                        t1[0:rows, :], pbf[0:rows, :], cos_sb[0:rows, tts],
                        MULT)
                    eng.tensor_tensor(
                        rt[0:rows, :], sg[0:rows, :], sin_sb[0:rows, tts],
                        MULT)
                    eng.tensor_tensor(
                        dest, t1[0:rows, :], rt[0:rows, :], ADD)

                def proj(rc, tt, name, slot):
                    """hi/lo DoubleRow projection: 12 accumulating DR matmuls
                    (4 K=256 blocks x {hi*hi, hi*lo, lo*hi}). slot borrows an
                    idle PSUM ring early in the prologue."""
                    tts = bass.ts(tt, TT)
                    pool, tag = slot
                    if tag == "sc":
                        ps = pool.tile([128, 2 * TT], F32, tag="sc",
                                       name=name)[:, 0:TT]
                    else:
                        ps = pool.tile([128, TT], F32, tag=tag, name=name)
                    steps = []
                    for j in range(4):
                        steps.append((whi_sb[:, rc, j], hhi_sb[:, j, :, tts]))
                    for j in range(4):
                        steps.append((whi_sb[:, rc, j], hlo_sb[:, j, :, tts]))
                        steps.append((wlo_sb[:, rc, j], hhi_sb[:, j, :, tts]))
                    for si, (w, x) in enumerate(steps):
                        nc.tensor.matmul(
                            ps[:], w, x,
                            start=(si == 0), stop=(si == len(steps) - 1),
                            perf_mode=DR)
                    return ps, tts

                def proj_kv(tt, slot=(psP, "proj"), act_copy=False):
                    ps, tts = proj(2, tt, f"projkv_{tt}", slot)
                    kbf = pbfp.tile([128, TT], BF, tag="pbf", name=f"kbf{tt}")
                    if act_copy:
                        nc.scalar.copy(kbf[0:64, :], ps[0:64, :])
                        nc.scalar.copy(vT[:, tts], ps[64:128, :])
                        nc.vector.tensor_scalar_mul(vT[:, tts], vT[:, tts],
                                                    1.0 / 16.0)
                    else:
                        nc.vector.tensor_copy(kbf[0:64, :], ps[0:64, :])
                        # v = ps/16
                        nc.vector.tensor_scalar_mul(vT[:, tts], ps[64:128, :],
                                                    1.0 / 16.0)
                    rope(kbf, k2[0:64, tts], 64, tts, "k")
                    nc.gpsimd.tensor_copy(k2[64:128, tts], k2[0:64, tts])

                def transp(tt, slot=(psP, "proj")):
                    pool, tag = slot
                    pt = pool.tile([128, 4, 64], BF, tag=tag,
                                   name=f"vt{tt}")
                    for ci in range(4):
                        c = 4 * tt + ci
                        nc.tensor.transpose(pt[:, ci, :],
                                            vT[:, bass.ts(c, 128)], ident[:])
                    # Act drains vaug: these run in the prologue window where
                    # Act has no exp work yet
                    if CFG["transp_eng"] == "act":
                        nc.scalar.copy(vaug[:, 4 * tt: 4 * tt + 4, 0:64],
                                       pt[:])
                    else:
                        nc.vector.tensor_copy(
                            vaug[:, 4 * tt: 4 * tt + 4, 0:64], pt[:])

                def proj_q(rc, tt, slot=(psP, "proj"), act_copy=False):
                    ps, tts = proj(rc, tt, f"projq{rc}_{tt}", slot)
                    pbf = pbfp.tile([128, TT], BF, tag="pbf",
                                    name=f"qbf{rc}_{tt}")
                    nc.vector.tensor_copy(pbf[:], ps[:])
                    # later q-tiles rope on Pool: they are pumped well ahead
                    # of their heads, and it keeps DVE free for PSUM drains
                    reng = nc.gpsimd if tt >= 1 else nc.vector
                    rope(pbf, qrot[rc][:, tts], 128, tts, "q", eng=reng)

                def outproj_tile(tch, ht, last=False, slot=(psP, "proj")):
                    tcs = bass.ts(tch, 128)
                    hts = bass.ts(ht, TT)
                    pool, tag = slot
                    if tag == "sc":
                        po = pool.tile([128, 2 * TT], F32, tag="sc",
                                       name=f"po{tch}_{ht}")[:, 0:TT]
                    else:
                        po = pool.tile([128, TT], F32, tag=tag,
                                       name=f"po{tch}_{ht}")
                    terms = [(ahi[:, :, tcs], wohi_sb[:, :, hts]),
                             (ahi[:, :, tcs], wolo_sb[:, :, hts]),
                             (alo[:, :, tcs], wohi_sb[:, :, hts])]
                    for si, (a, w) in enumerate(terms):
                        nc.tensor.matmul(
                            po[:], a, w,
                            start=(si == 0), stop=(si == len(terms) - 1),
                            perf_mode=DR)
                    ob = op_.tile([128, TT], BF, tag="ob")
                    # PSUM drain: only Act/DVE may read PSUM; DVE is the
                    # cheaper copier so it takes most of them
                    pat = CFG["ob_pat"]
                    k = ob_ctr[0]
                    ob_ctr[0] += 1
                    if pat == "rr3":
                        on_act = k % 3 == 1
                    elif pat == "alt2":
                        on_act = k % 2 == 1
                    else:
                        on_act = False
                    if on_act:
                        nc.scalar.copy(ob[:], po[:])
                    else:
                        nc.vector.tensor_copy(ob[:], po[:])
                    nc.sync.dma_start(out[tcs, hts], ob[:])

                # ---- filler queue
                filler = collections.deque()
                ob_ctr = [0]

                def pump(n=1):
                    for _ in range(n):
                        if not filler:
                            return
                        filler.popleft()()

                def attention_head(pair, h2, qt, exp_map=None,
                                   last_head=False):
                    if exp_map is None:
                        exp_map = {}
                    qts = bass.ts(qt, TT)
                    qrows = slice(64 * h2, 64 * h2 + 64)
                    pacc = psA.tile([128, 4, 65], F32, tag="att",
                                    name=f"att{pair}_{h2}_{qt}")
                    pending = collections.deque()

                    def drain_pending(keep):
                        # transposed attnV: the ex chunk is the STATIONARY
                        # operand and V+ones the moving one -> out free is 65
                        # (attn^T per 128-token chunk, denominator in col 64)
                        while len(pending) > keep:
                            pex, pcp = pending.popleft()
                            for j in range(2):
                                c = 2 * pcp + j
                                for qc in range(4):
                                    # the 4 qc accumulation groups share one
                                    # PSUM zero region: start/stop only once
                                    nc.tensor.matmul(
                                        pacc[:, qc, :],
                                        pex[:, 512 * j + 128 * qc:
                                            512 * j + 128 * qc + 128],
                                        vaug[:, c, :],
                                        start=(c == 0 and qc == 0),
                                        stop=(c == KC - 1 and qc == 3))

                    for cp in range(NCP):
                        sc = psS.tile([128, 2 * TT], F32, tag="sc",
                                      name=f"sc{pair}_{h2}_{qt}_{cp}")
                        for j in range(2):
                            c = 2 * cp + j
                            nc.tensor.matmul(
                                sc[:, bass.ts(j, TT)],
                                k2[qrows, bass.ts(c, 128)],
                                qrot[pair][qrows, qts],
                                start=True, stop=True,
                                tile_position=(64 * h2, 0))
                        if exp_map.get(cp) == "dve":
                            # offload this tile's exp to DVE (Schraudolph);
                            # ~2% approx error on those keys is within budget
                            # and relieves the pacing Activation engine.
                            # (Pool cannot read PSUM, so only DVE can assist.)
                            exi = ep.tile([128, 2 * TT], I16, tag="exps")
                            nc.vector.tensor_scalar(exi[:], sc[:],
                                                    SCH_A, SCH_B, MULT, ADD)
                            ex = exi[:].bitcast(BF)
                        else:
                            ext = ep.tile([128, 2 * TT], BF, tag="exp")
                            nc.scalar.activation(ext[:], sc[:], AF.Exp,
                                                 scale=SCALE)
                            ex = ext[:]
                        pending.append((ex, cp))
                        # attnV trails scores by two cpairs so the exp
                        # semaphore has always fired by the time the PE
                        # reaches the accumulation matmuls
                        drain_pending(CFG["keep_n"])
                        yield
                    drain_pending(0)
                    # attn^T normalize: the denominator is per-PARTITION ->
                    # Pool normalize_recip does the whole softmax divide;
                    # PE transposes back to [o, t] for the outproj
                    att = np_.tile([128, 4, 65], F32, tag="att_sb")
                    nc.vector.tensor_copy(att[:], pacc[:])
                    nrm = np_.tile([128, 4, 64], BF, tag="nrm")
                    for qc in range(4):
                        nc.gpsimd.normalize_recip(
                            nrm[:, qc, :], att[:, qc, 0:64],
                            att[:, qc, 64:65])
                    pt = psP.tile([64, 4, 128], BF, tag="proj",
                                  name=f"atp{pair}_{h2}_{qt}")
                    for qc in range(4):
                        nc.tensor.transpose(pt[:, qc, :], nrm[:, qc, :],
                                            ident128[:])
                    nc.vector.tensor_copy(anorm[qrows, pair, qts], pt[:])
                    # fp8 hi/lo for the DR outproj on Pool (all-SBUF; Pool is
                    # the idle engine at the tail)
                    nc.gpsimd.tensor_copy(ahi[qrows, pair, qts],
                                          anorm[qrows, pair, qts])
                    nc.gpsimd.tensor_tensor(
                        alo[qrows, pair, qts],
                        anorm[qrows, pair, qts],
                        ahi[qrows, pair, qts],
                        SUB)

                # ---- master schedule: kv0/q00 up front with Activation-
                # assisted copies (exp idle), V-transpose 0 borrows the psA
                # ring; later h tiles are DMA-gated so they pump as filler.
                def warm_mms(n, label):
                    for wi in range(n):
                        wps = psA.tile([128, TT], F32, tag="att",
                                       name=f"warm{label}_{wi}")
                        nc.tensor.matmul(wps[:, 0:256], wa[:, 0:128],
                                         wa[:, 0:256], start=True, stop=True)

                warm_mms(CFG["warm_n"], "a")
                # prologue projections spread across the still-idle psS banks
                # so consecutive projections never serialize on one PSUM bank
                proj_kv(0, slot=(psP, "proj"), act_copy=True)
                proj_q(0, 0, slot=(psS, "sc"), act_copy=True)
                transp(0, slot=(psA, "att"))
                proj_kv(1, slot=(psS, "sc"))
                q10_slot = ((psS, "sc") if CFG["q10_slot"] == "psS"
                            else (psA, "att"))
                filler.append(lambda: proj_kv(2, slot=(psS, "sc")))
                filler.append(lambda: transp(1))
                filler.append(lambda: proj_q(1, 0, slot=q10_slot))
                filler.append(lambda: proj_kv(3))
                filler.append(lambda: transp(2))
                filler.append(lambda: transp(3))
                for tt in range(1, NTT):
                    for rc in range(2):
                        filler.append(
                            lambda rc=rc, tt=tt: proj_q(rc, tt))

                heads = [(pair, h2, qt)
                         for qt in range(NTT)
                         for pair in range(2)
                         for h2 in range(2)]

                # tail (qt3) tiles alternate psP/psS PSUM slots so consecutive
                # po matmuls never wait on the previous tile's ob drain
                tail_ctr = [0]

                def tail_slot():
                    s = ((psP, "proj") if tail_ctr[0] % 2 == 0
                         else (psS, "sc"))
                    tail_ctr[0] += 1
                    return s

                def head_done(i):
                    if i % 4 == 3:
                        qt = heads[i][2]
                        last = i == len(heads) - 1
                        for tch in range(4 * qt, 4 * qt + 4):
                            for ht in range(2):
                                if qt == 3:
                                    filler.append(
                                        lambda tch=tch, ht=ht, last=last:
                                        outproj_tile(tch, ht, last=last,
                                                     slot=tail_slot()))
                                else:
                                    filler.append(
                                        lambda tch=tch, ht=ht, last=last:
                                        outproj_tile(tch, ht, last=last))

                nxt = 0
                # per-head exp engine map: Act is exact, DVE runs the
                # Schraudolph approximation. Act is the cheaper exp engine,
                # so DVE only takes what evens out the Act/DVE totals; the
                # middle heads get one extra (that is where the exp
                # backpressure stalls showed).
                def exp_map_for(i):
                    pat = CFG["exp_pat"]
                    if pat == "a":
                        if i == 0:
                            return {6: "dve"}
                        return {1: "dve", 3: "dve", 6: "dve"}
                    if pat == "b":
                        if i == 0:
                            return {6: "dve"}
                        if 4 <= i <= 11:
                            return {1: "dve", 3: "dve", 6: "dve"}
                        return {3: "dve", 6: "dve"}
                    if pat == "c":
                        return {1: "dve", 3: "dve", 6: "dve"}
                    if pat == "d":
                        if i == 0:
                            return {6: "dve"}
                        return {1: "dve", 4: "dve", 6: "dve"}
                    if pat == "e":
                        if i == 0:
                            return {6: "dve"}
                        return {2: "dve", 4: "dve", 6: "dve"}
                    if pat == "f":
                        if i == 0:
                            return {6: "dve"}
                        if i >= len(heads) - 2:
                            return {1: "dve", 3: "dve", 5: "dve", 6: "dve"}
                        return {1: "dve", 3: "dve", 6: "dve"}
                    raise ValueError(pat)

                def start_next():
                    nonlocal nxt
                    if nxt >= len(heads):
                        return None
                    g = attention_head(*heads[nxt],
                                       exp_map=exp_map_for(nxt),
                                       last_head=(nxt == len(heads) - 1))
                    nxt += 1
                    return (nxt - 1, g)

                nproj_fill = len(filler)
                slots = [start_next(), None]
                stagger = CFG["stagger"]
                step = 0
                while any(slots):
                    for si in range(2):
                        if slots[si] is None:
                            continue
                        i, g = slots[si]
                        try:
                            next(g)
                            # projection fillers drain at double rate (their
                            # PSUM->rope chains must stay ahead); outproj
                            # fillers at half rate so they cover the whole
                            # q-tile's rounds instead of bunching
                            if step < nproj_fill:
                                pump(2)
                            elif step % CFG["pump_div"] == 0:
                                pump(1)
                            step += 1
                            if stagger is not None:
                                stagger -= 1
                                if stagger == 0:
                                    slots[1] = start_next()
                                    stagger = None
                        except StopIteration:
                            head_done(i)
                            slots[si] = start_next()
                # bridge the last norm chain with junk matmuls so the
                # final outproj tiles run at full PE clock
                filler.appendleft(lambda: warm_mms(10, "t"))
                while filler:
                    pump(1)
    nc.finalize()
    return nc


def _get_nc():
    global _nc_cache
    if _nc_cache is None:
        _nc_cache = _build_bass()
    return _nc_cache


def _hilo(x):
    hi = x.astype(_E4)
    lo = (x - hi.astype(np.float32)).astype(_E5)
    return hi, lo


def _shard_inputs(hidden_states, cos, sin, w_qkv, w_o):
    """Build per-core input maps. Core c = (b = c // 4, g = c % 4)."""
    cosT = cos.T.astype(np.float32)                                # [64, S]
    sinT = sin.T.astype(np.float32)
    sinmod = np.concatenate([-sinT[0:32], sinT[32:64]], axis=0)
    cosc = np.ascontiguousarray(cosT / 16.0).astype(_BF16)
    sinc = np.ascontiguousarray(sinmod / 16.0).astype(_BF16)

    # h packed [p, j, i, t]: feature 256j + 128i + p
    hsplit = []
    for b in range(B):
        ht = hidden_states[b].T.astype(np.float32)                 # [1024, S]
        hp = np.ascontiguousarray(
            ht.reshape(4, 2, 128, S).transpose(2, 0, 1, 3))        # [128,4,2,S]
        hsplit.append(_hilo(hp))
    in_maps = []
    for c in range(NCORES):
        b, g = divmod(c, 4)
        q_rows = w_qkv[256 * g: 256 * g + 256]
        k_rows = w_qkv[1024 + 64 * g: 1024 + 64 * g + 64]
        v_rows = w_qkv[1280 + 64 * g: 1280 + 64 * g + 64]
        wqk = np.concatenate([q_rows, k_rows, v_rows], axis=0)     # [384, 1024]
        # x16 into fp8 range; [p, rc, j, i, m] with h = 256j+128i+p
        wqkT = (wqk.T * 16.0).astype(np.float32)                   # [1024, 384]
        wpk = np.ascontiguousarray(
            wqkT.reshape(4, 2, 128, 3, 128).transpose(2, 3, 0, 1, 4))
        whi_a, wlo_a = _hilo(wpk)
        woTf = (w_o[:, 256 * g: 256 * g + 256].T * 16.0).astype(np.float32)
        wo_pk = np.ascontiguousarray(
            woTf.reshape(2, 128, HID).transpose(1, 0, 2))          # [128,2,HID]
        wohi_a, wolo_a = _hilo(wo_pk)
        in_maps.append(
            {
                "hhi": hsplit[b][0],
                "hlo": hsplit[b][1],
                "whi": whi_a,
                "wlo": wlo_a,
                "wohi": wohi_a,
                "wolo": wolo_a,
                "cosd": cosc,
                "sind": sinc,
            }
        )
    return in_maps


def _run(inputs, **spmd_kwargs):
    from concourse.bass_utils import run_bass_kernel_spmd

    nc = _get_nc()
    in_maps = _shard_inputs(**inputs)
    res = run_bass_kernel_spmd(
        nc, in_maps, core_ids=list(range(NCORES)), **spmd_kwargs
    )
    outs = []
    for b in range(B):
        acc = res.results[4 * b]["out"].astype(np.float32)
        for g in range(1, 4):
            acc = acc + res.results[4 * b + g]["out"].astype(np.float32)
        outs.append(acc * OUT_SCALE)
    return np.stack(outs, axis=0), res


def kernel(**inputs):
    out, _ = _run(inputs)
    return out



# revision 59
# speedup vs baseline: 1.0627x; 1.0012x over previous
"""GQA attention layer (QKV proj + RoPE + softmax attention + out proj) on 8
Trainium2 NeuronCores.

Sharding: core c = (batch b = c//4, head-group g = c%4): 4 q heads + 1 kv
head per core, w_o row-parallel partial output in bf16; the host upcasts,
sums the 4 partials per batch and divides by the fp8 scaling factor (512).

Design highlights vs the original baseline (234 us -> 164 us):
- fp8 hi/lo DoubleRow matmuls for both projections: operands split into
  e4m3 hi + e5m2 lo (host-side for h/w_qkv/w_o, Pool-side for the attention
  output); each K=256 block runs as 3 DR matmuls (hi*hi + hi*lo + lo*hi) at
  0.75x the bf16 PE cost with ~bf16 accuracy. scores/attnV stay bf16
  (single-fp8 would blow the 2e-2 error budget).
- SCALE folded into the exp activation so q and k share one compact rope
  table; exp runs on Activation in [128, 1024] tiles; 1-2 tiles per head
  (~18% of keys) offload to DVE via a Schraudolph bitcast exp
  (int16(A*s + B) reinterpreted as bf16), weighted toward rounds where the
  PE has no filler work.
- softmax normalize: pacc copied off PSUM immediately (the psA bank recycles
  in one op), reciprocal on a partition-0 staged denominator (PSUM-sourced
  or partition-offset reciprocal inputs return garbage on HW), Pool
  partition_broadcast + Pool multiply replace the baseline's fp32 PE
  broadcast matmuls (everything SBUF-side is Pool-legal).
- schedule: two staggered attention-head generators with a filler queue
  (projections at 2x pump rate, V transposes, outproj tiles at 1/4 rate to
  cover whole q-tiles); junk warm-up matmuls bridge the initial DMA wait and
  the final norm chain so the PE p-state never drops mid-kernel; DMA order
  tuned so each k/v tile lands just before the attention wavefront needs it.
"""

import collections

import numpy as np
import ml_dtypes

B, S, HID = 2, 2048, 1024
NH, NKV, D = 16, 4, 64
SCALE = float(D ** -0.5)
NCORES = 8
TT = 512          # token tile
NTT = S // TT     # 4
KC = S // 128     # 16 key chunks
NCP = KC // 2     # 8 chunk pairs
OUT_SCALE = 1.0 / 512.0

_BF16 = ml_dtypes.bfloat16
_E4 = ml_dtypes.float8_e4m3
_E5 = ml_dtypes.float8_e5m2

_nc_cache = None

# schedule knobs (tuned by simulation sweep; defaults are the shipped config)
CFG = {
    "transp_eng": "act",   # engine for the vaug drain after V transposes
    "q10_slot": "psA",     # PSUM slot for the prologue proj_q(1, 0)
    "dma_hlo_early": True,  # issue hlo tile0 before wlo2
    "warm_n": 22,           # prologue warm-up matmuls
    "ob_pat": "alt2",       # outproj PSUM-drain engine pattern
    "stagger": 12,          # rounds before the second head slot starts
    "exp_pat": "a",         # which chunk-pairs Schraudolph on DVE per head
    "pump_div": 2,          # outproj filler pump cadence (1 per N rounds)
    "keep_n": 6,            # attnV trails scores by keep_n chunk-pairs
    "q00_pbf_act": True,    # drain q00 PSUM on Act (idle in prologue)
    "tail_warm": 0,         # junk matmuls before the final outproj (unused)
}


def _build_bass():
    import concourse.bass as bass
    import concourse.mybir as mybir
    import concourse.tile as tile
    from concourse import bacc
    from concourse.masks import make_identity

    BF = mybir.dt.bfloat16
    F32 = mybir.dt.float32
    I16 = mybir.dt.int16
    E4 = mybir.dt.float8e4
    E5 = mybir.dt.float8e5
    AF = mybir.ActivationFunctionType
    MULT = mybir.AluOpType.mult
    ADD = mybir.AluOpType.add
    SUB = mybir.AluOpType.subtract
    DR = mybir.MatmulPerfMode.DoubleRow
    # Schraudolph exp for offloaded tiles: bitcast(int16(s*A + B)) ~ exp(s)
    SCH_A = 184.66496280558537 * SCALE   # 128/ln2, scores carry 1/SCALE
    SCH_B = 16256.0 - 5.75 + 0.5         # bias center + truncation fix

    nc = bacc.Bacc()
    # h split hi/lo, packed [p, j, i, t]: h feature 256j + 128i + p
    hhi = nc.dram_tensor("hhi", (128, 4, 2, S), E4, kind="ExternalInput")
    hlo = nc.dram_tensor("hlo", (128, 4, 2, S), E5, kind="ExternalInput")
    # wqk split hi/lo, packed [p, rc, j, i, m]
    whi = nc.dram_tensor("whi", (128, 3, 4, 2, 128), E4, kind="ExternalInput")
    wlo = nc.dram_tensor("wlo", (128, 3, 4, 2, 128), E5, kind="ExternalInput")
    wohi = nc.dram_tensor("wohi", (128, 2, HID), E4, kind="ExternalInput")
    wolo = nc.dram_tensor("wolo", (128, 2, HID), E5, kind="ExternalInput")
    # shared q/k rope tables (/16); SCALE is applied by the exp activation
    cosd = nc.dram_tensor("cosd", (64, S), BF, kind="ExternalInput")
    sind = nc.dram_tensor("sind", (64, S), BF, kind="ExternalInput")
    out = nc.dram_tensor("out", (S, HID), BF, kind="ExternalOutput")

    with tile.TileContext(nc) as tc:
        with (
            tc.tile_pool(name="persist", bufs=1) as pp,
            tc.tile_pool(name="pbfp", bufs=3) as pbfp,
            tc.tile_pool(name="rope", bufs=3) as rp,
            tc.tile_pool(name="exps", bufs=CFG.get("ep_bufs", 6)) as ep,
            tc.tile_pool(name="norm", bufs=4) as np_,
            tc.tile_pool(name="outsb", bufs=4) as op_,
        ):
            # ---- persistent SBUF tiles + input loads, kv-first order
            hhi_sb = pp.tile([128, 4, 2, S], E4, tag="hhi_sb")
            hlo_sb = pp.tile([128, 4, 2, S], E5, tag="hlo_sb")
            whi_sb = pp.tile([128, 3, 4, 2, 128], E4, tag="whi_sb")
            wlo_sb = pp.tile([128, 3, 4, 2, 128], E5, tag="wlo_sb")
            wohi_sb = pp.tile([128, 2, HID], E4, tag="wohi_sb")
            wolo_sb = pp.tile([128, 2, HID], E5, tag="wolo_sb")
            # [128, S]: rows 0:64 DMA'd, rows 64:128 duplicated on-device
            cos_sb = pp.tile([128, S], BF, tag="cos_sb")
            sin_sb = pp.tile([128, S], BF, tag="sin_sb")

            def h_slice(t0, t1):
                for hd, hs in ((hhi, hhi_sb), (hlo, hlo_sb)):
                    nc.sync.dma_start(hs[:, :, :, t0:t1], hd[:, :, :, t0:t1])

            def cossin(tt):
                tts_ = bass.ts(tt, TT)
                nc.sync.dma_start(cos_sb[0:64, tts_], cosd[:, tts_])
                nc.sync.dma_start(sin_sb[0:64, tts_], sind[:, tts_])
                # Pool duplicates the tables onto partitions 64:128
                nc.gpsimd.tensor_copy(cos_sb[64:128, tts_],
                                      cos_sb[0:64, tts_])
                nc.gpsimd.tensor_copy(sin_sb[64:128, tts_],
                                      sin_sb[0:64, tts_])

            # prologue DMA order: hi-dtype pieces first so the interleaved
            # hi-step matmuls can start while the lo pieces stream in
            dma_ops = {
                "whi2": lambda: nc.sync.dma_start(whi_sb[:, 2], whi[:, 2]),
                "hhi02": lambda: nc.sync.dma_start(
                    hhi_sb[:, 0:2, :, 0:TT], hhi[:, 0:2, :, 0:TT]),
                "hhi24": lambda: nc.sync.dma_start(
                    hhi_sb[:, 2:4, :, 0:TT], hhi[:, 2:4, :, 0:TT]),
                "hlo": lambda: nc.sync.dma_start(
                    hlo_sb[:, :, :, 0:TT], hlo[:, :, :, 0:TT]),
                "wlo2": lambda: nc.sync.dma_start(wlo_sb[:, 2], wlo[:, 2]),
                "whi01": lambda: nc.sync.dma_start(
                    whi_sb[:, 0:2], whi[:, 0:2]),
                "wlo01": lambda: nc.sync.dma_start(
                    wlo_sb[:, 0:2], wlo[:, 0:2]),
                "cos0": lambda: cossin(0),
            }
            for tt in range(1, NTT):
                dma_ops[f"h{tt}"] = (
                    lambda tt=tt: h_slice(tt * TT, (tt + 1) * TT))
                dma_ops[f"cos{tt}"] = lambda tt=tt: cossin(tt)
            dma_ops["wo"] = lambda: (
                nc.sync.dma_start(wohi_sb[:], wohi[:]),
                nc.sync.dma_start(wolo_sb[:], wolo[:]))
            orders = {
                "v0": ["whi2", "hhi02", "hhi24", "hlo", "wlo2", "cos0",
                       "whi01", "wlo01",
                       "h1", "cos1", "h2", "cos2", "h3", "cos3", "wo"],
                "v6": ["whi2", "hhi02", "hhi24", "hlo", "wlo2", "cos0",
                       "h1", "whi01", "wlo01",
                       "cos1", "h2", "cos2", "h3", "cos3", "wo"],
                "v7": ["whi2", "hhi02", "hhi24", "hlo", "wlo2", "cos0",
                       "h1", "h2", "whi01", "wlo01",
                       "cos1", "cos2", "h3", "cos3", "wo"],
                "v8": ["whi2", "hhi02", "hhi24", "hlo", "wlo2", "cos0",
                       "whi01", "h1", "wlo01",
                       "cos1", "h2", "cos2", "h3", "cos3", "wo"],
                "v9": ["whi2", "hhi02", "hhi24", "hlo", "wlo2", "cos0",
                       "whi01", "wlo01",
                       "h1", "h2", "h3", "cos1", "cos2", "cos3", "wo"],
                "v10": ["whi2", "hhi02", "hhi24", "hlo", "wlo2", "cos0",
                        "whi01", "wlo01",
                        "h1", "h2", "cos1", "h3", "cos2", "cos3", "wo"],
            }
            for op in orders[CFG.get("dma_order", "v0")]:
                dma_ops[op]()

            ident = pp.tile([64, 64], BF, tag="ident")
            make_identity(nc, ident[:])
            ident128 = pp.tile([128, 128], BF, tag="ident128")
            make_identity(nc, ident128[:])
            warm = pp.tile([1, 8], F32, tag="warm")
            nc.any.memset(warm[:], 0.0)
            nc.scalar.activation(warm[:], warm[:], AF.Exp)
            # keep the tensor engine busy on junk matmuls while the first
            # h/w DMAs land, so the p-state ramp completes before real work
            wa = pp.tile([128, TT], BF, tag="wa")
            nc.gpsimd.memset(wa[:], 0.5)

            qrot = [pp.tile([128, S], BF, tag=f"qrot{p}", name=f"qrot{p}")
                    for p in range(2)]
            k2 = pp.tile([128, S], BF, tag="k2")
            vT = pp.tile([64, S], BF, tag="vT")
            vaug = pp.tile([128, KC, 65], BF, tag="vaug")
            nc.any.memset(vaug[:], 1.0 / 32.0)
            # anorm = 32*attn: bf16 full + fp8 hi/lo for the DR outproj,
            # packed [p, oc(=pair), t]
            anorm = pp.tile([128, 2, S], BF, tag="anorm")
            ahi = pp.tile([128, 2, S], E4, tag="ahi")
            alo = pp.tile([128, 2, S], E5, tag="alo")

            with (
                tc.tile_pool(name="psP", bufs=1, space="PSUM") as psP,
                tc.tile_pool(name="psS", bufs=3, space="PSUM") as psS,
                tc.tile_pool(name="psA", bufs=1, space="PSUM") as psA,
            ):

                def rope(pbf, dest, rows, tts, tag, eng=None):
                    """Pool builds the 32-block-swapped copy; the multiply/add
                    ops (all-SBUF bf16) default to DVE 2x mode but can run on
                    Pool when DVE is the busier engine."""
                    if eng is None:
                        eng = nc.vector
                    sg = rp.tile([128, TT], BF, tag=f"sg{tag}")
                    for blk in range(rows // 32):
                        src = blk ^ 1
                        nc.gpsimd.tensor_copy(
                            sg[32 * blk: 32 * blk + 32, :],
                            pbf[32 * src: 32 * src + 32, :])
                    t1 = rp.tile([128, TT], BF, tag=f"t1{tag}")
                    rt = rp.tile([128, TT], BF, tag=f"rt{tag}")
                    eng.tensor_tensor(
                        t1[0:rows, :], pbf[0:rows, :], cos_sb[0:rows, tts],
                        MULT)
                    eng.tensor_tensor(
                        rt[0:rows, :], sg[0:rows, :], sin_sb[0:rows, tts],
                        MULT)
                    eng.tensor_tensor(
                        dest, t1[0:rows, :], rt[0:rows, :], ADD)

                def proj_alloc(tt, name, slot):
                    tts = bass.ts(tt, TT)
                    pool, tag = slot
                    if tag == "sc":
                        ps = pool.tile([128, 2 * TT], F32, tag="sc",
                                       name=name)[:, 0:TT]
                    else:
                        ps = pool.tile([128, TT], F32, tag=tag, name=name)
                    return ps, tts

                def proj_steps(rc, tts, ps, phase=None):
                    """hi/lo DoubleRow projection: 12 accumulating DR matmuls
                    (4 K=256 blocks x {hi*hi, hi*lo, lo*hi}). phase="hi" emits
                    only the 4 hi*hi steps (start), "lo" the 8 mixed ones
                    (stop) -- used to interleave two DMA-gated projections in
                    the prologue."""
                    hi = [(whi_sb[:, rc, j], hhi_sb[:, j, :, tts])
                          for j in range(4)]
                    lo = []
                    for j in range(4):
                        lo.append((whi_sb[:, rc, j], hlo_sb[:, j, :, tts]))
                        lo.append((wlo_sb[:, rc, j], hhi_sb[:, j, :, tts]))
                    if phase == "hi":
                        steps, start, stop = hi, True, False
                    elif phase == "lo":
                        steps, start, stop = lo, False, True
                    else:
                        steps, start, stop = hi + lo, True, True
                    for si, (w, x) in enumerate(steps):
                        nc.tensor.matmul(
                            ps[:], w, x,
                            start=(start and si == 0),
                            stop=(stop and si == len(steps) - 1),
                            perf_mode=DR)

                def proj(rc, tt, name, slot):
                    ps, tts = proj_alloc(tt, name, slot)
                    proj_steps(rc, tts, ps)
                    return ps, tts

                def proj_kv_finish(ps, tts, tt, act_copy=False):
                    kbf = pbfp.tile([128, TT], BF, tag="pbf", name=f"kbf{tt}")
                    if act_copy:
                        nc.scalar.copy(kbf[0:64, :], ps[0:64, :])
                        nc.scalar.copy(vT[:, tts], ps[64:128, :])
                        nc.vector.tensor_scalar_mul(vT[:, tts], vT[:, tts],
                                                    1.0 / 16.0)
                    else:
                        nc.vector.tensor_copy(kbf[0:64, :], ps[0:64, :])
                        # v = ps/16
                        nc.vector.tensor_scalar_mul(vT[:, tts], ps[64:128, :],
                                                    1.0 / 16.0)
                    rope(kbf, k2[0:64, tts], 64, tts, "k")
                    nc.gpsimd.tensor_copy(k2[64:128, tts], k2[0:64, tts])

                def proj_kv(tt, slot=(psP, "proj"), act_copy=False):
                    ps, tts = proj(2, tt, f"projkv_{tt}", slot)
                    proj_kv_finish(ps, tts, tt, act_copy=act_copy)

                def transp(tt, slot=(psP, "proj")):
                    pool, tag = slot
                    pt = pool.tile([128, 4, 64], BF, tag=tag,
                                   name=f"vt{tt}")
                    for ci in range(4):
                        c = 4 * tt + ci
                        nc.tensor.transpose(pt[:, ci, :],
                                            vT[:, bass.ts(c, 128)], ident[:])
                    # Act drains vaug: these run in the prologue window where
                    # Act has no exp work yet
                    if CFG["transp_eng"] == "act":
                        nc.scalar.copy(vaug[:, 4 * tt: 4 * tt + 4, 0:64],
                                       pt[:])
                    else:
                        nc.vector.tensor_copy(
                            vaug[:, 4 * tt: 4 * tt + 4, 0:64], pt[:])

                def proj_q_finish(ps, tts, rc, tt):
                    pbf = pbfp.tile([128, TT], BF, tag="pbf",
                                    name=f"qbf{rc}_{tt}")
                    if tt == 0 and rc == 0 and CFG.get("q00_pbf_act", False):
                        nc.scalar.copy(pbf[:], ps[:])
                    else:
                        nc.vector.tensor_copy(pbf[:], ps[:])
                    # later q-tiles rope on Pool: they are pumped well ahead
                    # of their heads, and it keeps DVE free for PSUM drains
                    reng = nc.gpsimd if tt >= 1 else nc.vector
                    rope(pbf, qrot[rc][:, tts], 128, tts, "q", eng=reng)

                def proj_q(rc, tt, slot=(psP, "proj"), act_copy=False):
                    ps, tts = proj(rc, tt, f"projq{rc}_{tt}", slot)
                    proj_q_finish(ps, tts, rc, tt)

                def ob_drain(ob_sl, po_sl, dram_sl):
                    # PSUM drain: only Act/DVE may read PSUM; DVE is the
                    # cheaper copier so it takes most of them
                    pat = CFG["ob_pat"]
                    k = ob_ctr[0]
                    ob_ctr[0] += 1
                    if pat == "rr3":
                        on_act = k % 3 == 1
                    elif pat == "alt2":
                        on_act = k % 2 == 1
                    else:
                        on_act = False
                    if on_act:
                        nc.scalar.copy(ob_sl, po_sl)
                    else:
                        nc.vector.tensor_copy(ob_sl, po_sl)
                    nc.sync.dma_start(dram_sl, ob_sl)

                def outproj_tile(tch, ht, last=False, slot=(psP, "proj"),
                                 split=False):
                    tcs = bass.ts(tch, 128)
                    pool, tag = slot
                    if tag == "sc":
                        po = pool.tile([128, 2 * TT], F32, tag="sc",
                                       name=f"po{tch}_{ht}")[:, 0:TT]
                    else:
                        po = pool.tile([128, TT], F32, tag=tag,
                                       name=f"po{tch}_{ht}")
                    ob = op_.tile([128, TT], BF, tag="ob")
                    # split=True emits two half-width accumulation groups so
                    # the drain+DMA pipeline runs at twice the granularity
                    # (same PE rows; used for the tail tiles)
                    nh = 2 if split else 1
                    for hh in range(nh):
                        h0 = ht * TT + hh * (TT // nh)
                        hts = slice(h0, h0 + TT // nh)
                        pos = slice(hh * (TT // nh), (hh + 1) * (TT // nh))
                        terms = [(ahi[:, :, tcs], wohi_sb[:, :, hts]),
                                 (ahi[:, :, tcs], wolo_sb[:, :, hts]),
                                 (alo[:, :, tcs], wohi_sb[:, :, hts])]
                        for si, (a, w) in enumerate(terms):
                            nc.tensor.matmul(
                                po[:, pos], a, w,
                                start=(si == 0), stop=(si == len(terms) - 1),
                                perf_mode=DR)
                        ob_drain(ob[:, pos], po[:, pos], out[tcs, hts])

                # ---- filler queue
                filler = collections.deque()
                ob_ctr = [0]

                def pump(n=1):
                    for _ in range(n):
                        if not filler:
                            return
                        filler.popleft()()

                def attention_head(pair, h2, qt, exp_map=None,
                                   last_head=False, keep_n=None):
                    if exp_map is None:
                        exp_map = {}
                    if keep_n is None:
                        keep_n = CFG["keep_n"]
                    qts = bass.ts(qt, TT)
                    qrows = slice(64 * h2, 64 * h2 + 64)
                    pacc = psA.tile([128, 4, 65], F32, tag="att",
                                    name=f"att{pair}_{h2}_{qt}")
                    pending = collections.deque()

                    def drain_pending(keep):
                        # transposed attnV: the ex chunk is the STATIONARY
                        # operand and V+ones the moving one -> out free is 65
                        # (attn^T per 128-token chunk, denominator in col 64)
                        while len(pending) > keep:
                            pex, pcp = pending.popleft()
                            for j in range(2):
                                c = 2 * pcp + j
                                for qc in range(4):
                                    # the 4 qc accumulation groups share one
                                    # PSUM zero region: start/stop only once
                                    nc.tensor.matmul(
                                        pacc[:, qc, :],
                                        pex[:, 512 * j + 128 * qc:
                                            512 * j + 128 * qc + 128],
                                        vaug[:, c, :],
                                        start=(c == 0 and qc == 0),
                                        stop=(c == KC - 1 and qc == 3))

                    for cp in range(NCP):
                        sc = psS.tile([128, 2 * TT], F32, tag="sc",
                                      name=f"sc{pair}_{h2}_{qt}_{cp}")
                        for j in range(2):
                            c = 2 * cp + j
                            nc.tensor.matmul(
                                sc[:, bass.ts(j, TT)],
                                k2[qrows, bass.ts(c, 128)],
                                qrot[pair][qrows, qts],
                                start=True, stop=True,
                                tile_position=(64 * h2, 0))
                        if exp_map.get(cp) == "dve":
                            # offload this tile's exp to DVE (Schraudolph);
                            # ~2% approx error on those keys is within budget
                            # and relieves the pacing Activation engine.
                            # (Pool cannot read PSUM, so only DVE can assist.)
                            exi = ep.tile([128, 2 * TT], I16, tag="exps")
                            nc.vector.tensor_scalar(exi[:], sc[:],
                                                    SCH_A, SCH_B, MULT, ADD)
                            ex = exi[:].bitcast(BF)
                        else:
                            ext = ep.tile([128, 2 * TT], BF, tag="exp")
                            nc.scalar.activation(ext[:], sc[:], AF.Exp,
                                                 scale=SCALE)
                            ex = ext[:]
                        pending.append((ex, cp))
                        # attnV trails scores by two cpairs so the exp
                        # semaphore has always fired by the time the PE
                        # reaches the accumulation matmuls
                        drain_pending(keep_n)
                        yield
                    drain_pending(0)
                    # attn^T normalize: the denominator is per-PARTITION ->
                    # Pool normalize_recip does the whole softmax divide;
                    # PE transposes back to [o, t] for the outproj
                    att = np_.tile([128, 4, 65], F32, tag="att_sb")
                    nc.vector.tensor_copy(att[:], pacc[:])
                    nrm = np_.tile([128, 4, 64], BF, tag="nrm")
                    for qc in range(4):
                        nc.gpsimd.normalize_recip(
                            nrm[:, qc, :], att[:, qc, 0:64],
                            att[:, qc, 64:65])
                    # the last head's transpose borrows the freed psA bank so
                    # the tail outproj tiles never queue behind it on psP
                    atp_on_a = last_head and CFG.get("atp_psa", True)
                    atp_pool, atp_tag = (psA, "att") if atp_on_a \
                        else (psP, "proj")
                    pt = atp_pool.tile([64, 4, 128], BF, tag=atp_tag,
                                       name=f"atp{pair}_{h2}_{qt}")
                    for qc in range(4):
                        nc.tensor.transpose(pt[:, qc, :], nrm[:, qc, :],
                                            ident128[:])
                    nc.vector.tensor_copy(anorm[qrows, pair, qts], pt[:])
                    # fp8 hi/lo for the DR outproj on Pool (all-SBUF; Pool is
                    # the idle engine at the tail)
                    nc.gpsimd.tensor_copy(ahi[qrows, pair, qts],
                                          anorm[qrows, pair, qts])
                    nc.gpsimd.tensor_tensor(
                        alo[qrows, pair, qts],
                        anorm[qrows, pair, qts],
                        ahi[qrows, pair, qts],
                        SUB)

                # ---- master schedule: kv0/q00 up front with Activation-
                # assisted copies (exp idle), V-transpose 0 borrows the psA
                # ring; later h tiles are DMA-gated so they pump as filler.
                def warm_mms(n, label):
                    for wi in range(n):
                        wps = psA.tile([128, TT], F32, tag="att",
                                       name=f"warm{label}_{wi}")
                        nc.tensor.matmul(wps[:, 0:256], wa[:, 0:128],
                                         wa[:, 0:256], start=True, stop=True)

                warm_mms(CFG["warm_n"], "a")
                # prologue projections spread across the still-idle psS banks
                # so consecutive projections never serialize on one PSUM bank.
                # kv0 and q00 interleave their hi and lo step groups: the hi
                # steps only need the hi-dtype DMAs (which land first), so the
                # PE isn't stuck in-order behind kv0's lo steps waiting on hlo
                if CFG.get("interleave_prologue", True):
                    ps_kv, tts0 = proj_alloc(0, "projkv_0", (psP, "proj"))
                    ps_q, _ = proj_alloc(0, "projq0_0", (psS, "sc"))
                    proj_steps(2, tts0, ps_kv, "hi")
                    proj_steps(0, tts0, ps_q, "hi")
                    proj_steps(2, tts0, ps_kv, "lo")
                    proj_steps(0, tts0, ps_q, "lo")
                    proj_kv_finish(ps_kv, tts0, 0, act_copy=True)
                    proj_q_finish(ps_q, tts0, 0, 0)
                else:
                    proj_kv(0, slot=(psP, "proj"), act_copy=True)
                    proj_q(0, 0, slot=(psS, "sc"), act_copy=True)
                transp(0, slot=(psA, "att"))
                proj_kv(1, slot=(psS, "sc"))
                q10_slot = ((psS, "sc") if CFG["q10_slot"] == "psS"
                            else (psA, "att"))
                filler.append(lambda: proj_kv(2, slot=(psS, "sc")))
                filler.append(lambda: transp(1))
                filler.append(lambda: proj_q(1, 0, slot=q10_slot))
                filler.append(lambda: proj_kv(3))
                filler.append(lambda: transp(2))
                filler.append(lambda: transp(3))
                for tt in range(1, NTT):
                    for rc in range(2):
                        filler.append(
                            lambda rc=rc, tt=tt: proj_q(rc, tt))

                heads = [(pair, h2, qt)
                         for qt in range(NTT)
                         for pair in range(2)
                         for h2 in range(2)]

                # tail (qt3) tiles rotate over 4 PSUM slots (psP + the three
                # psS bufs, idle once scores end) so po matmuls never wait on
                # an ob drain: with 4 slots in flight the ~600ns copy hides
                # behind 4 x 320ns of matmuls
                tail_ctr = [0]

                def tail_slot():
                    s = ((psP, "proj") if tail_ctr[0] % 4 == 0
                         else (psS, "sc"))
                    tail_ctr[0] += 1
                    return s

                def head_done(i):
                    if i % 4 == 3:
                        qt = heads[i][2]
                        last = i == len(heads) - 1
                        for tch in range(4 * qt, 4 * qt + 4):
                            for ht in range(2):
                                if qt == 3:
                                    filler.append(
                                        lambda tch=tch, ht=ht, last=last:
                                        outproj_tile(
                                            tch, ht, last=last,
                                            slot=tail_slot(),
                                            split=CFG.get("split_tail",
                                                          False)))
                                else:
                                    filler.append(
                                        lambda tch=tch, ht=ht, last=last:
                                        outproj_tile(tch, ht, last=last))

                nxt = 0
                # per-head exp engine map: Act is exact, DVE runs the
                # Schraudolph approximation. Act is the cheaper exp engine,
                # so DVE only takes what evens out the Act/DVE totals; the
                # middle heads get one extra (that is where the exp
                # backpressure stalls showed).
                def exp_map_for(i):
                    pat = CFG["exp_pat"]
                    if pat == "a":
                        if i == 0:
                            return {6: "dve"}
                        return {1: "dve", 3: "dve", 6: "dve"}
                    if pat == "b":
                        if i == 0:
                            return {6: "dve"}
                        if 4 <= i <= 11:
                            return {1: "dve", 3: "dve", 6: "dve"}
                        return {3: "dve", 6: "dve"}
                    if pat == "c":
                        return {1: "dve", 3: "dve", 6: "dve"}
                    if pat == "d":
                        if i == 0:
                            return {6: "dve"}
                        return {1: "dve", 4: "dve", 6: "dve"}
                    if pat == "e":
                        if i == 0:
                            return {6: "dve"}
                        return {2: "dve", 4: "dve", 6: "dve"}
                    if pat == "f":
                        if i == 0:
                            return {6: "dve"}
                        if i >= len(heads) - 2:
                            return {1: "dve", 3: "dve", 5: "dve", 6: "dve"}
                        return {1: "dve", 3: "dve", 6: "dve"}
                    if pat == "g":
                        if i == 0:
                            return {3: "dve", 6: "dve"}
                        return {1: "dve", 3: "dve", 5: "dve", 6: "dve"}
                    if pat == "h":
                        if i == 0:
                            return {6: "dve"}
                        if 4 <= i <= 11:
                            return {1: "dve", 3: "dve", 5: "dve", 6: "dve"}
                        return {1: "dve", 3: "dve", 6: "dve"}
                    raise ValueError(pat)

                def start_next():
                    nonlocal nxt
                    if nxt >= len(heads):
                        return None
                    is_late = nxt >= len(heads) - 2
                    em = dict(exp_map_for(nxt))
                    if is_late and CFG.get("last_cp7_dve", False):
                        em[7] = "dve"
                        em.pop(6, None)  # keep the approx-tile count level
                    g = attention_head(*heads[nxt],
                                       exp_map=em,
                                       last_head=(nxt == len(heads) - 1),
                                       keep_n=(CFG.get("keep_last")
                                               if is_late else None))
                    nxt += 1
                    return (nxt - 1, g)

                nproj_fill = len(filler)
                n_slots = CFG.get("n_slots", 2)
                slots = [start_next()] + [None] * (n_slots - 1)
                stagger = CFG["stagger"]
                next_slot = 1
                countdown = stagger
                step = 0
                while any(slots):
                    for si in range(n_slots):
                        if slots[si] is None:
                            continue
                        i, g = slots[si]
                        try:
                            next(g)
                            # projection fillers drain at double rate (their
                            # PSUM->rope chains must stay ahead); outproj
                            # fillers at half rate so they cover the whole
                            # q-tile's rounds instead of bunching
                            if step < nproj_fill:
                                pump(2)
                            elif step % CFG["pump_div"] == 0:
                                pump(1)
                            step += 1
                            if next_slot < n_slots:
                                countdown -= 1
                                if countdown == 0:
                                    slots[next_slot] = start_next()
                                    next_slot += 1
                                    countdown = stagger
                        except StopIteration:
                            head_done(i)
                            slots[si] = start_next()
                # (no tail warm bridge needed: the cost model's p-state never
                # downclocks once ramped, so idle before the final outproj
                # tiles is free)
                if CFG.get("tail_warm", 0):
                    filler.appendleft(
                        lambda: warm_mms(CFG["tail_warm"], "t"))
                while filler:
                    pump(1)
    nc.finalize()
    return nc


def _get_nc():
    global _nc_cache
    if _nc_cache is None:
        _nc_cache = _build_bass()
    return _nc_cache


def _hilo(x):
    hi = x.astype(_E4)
    lo = (x - hi.astype(np.float32)).astype(_E5)
    return hi, lo


def _shard_inputs(hidden_states, cos, sin, w_qkv, w_o):
    """Build per-core input maps. Core c = (b = c // 4, g = c % 4)."""
    cosT = cos.T.astype(np.float32)                                # [64, S]
    sinT = sin.T.astype(np.float32)
    sinmod = np.concatenate([-sinT[0:32], sinT[32:64]], axis=0)
    cosc = np.ascontiguousarray(cosT / 16.0).astype(_BF16)
    sinc = np.ascontiguousarray(sinmod / 16.0).astype(_BF16)

    # h packed [p, j, i, t]: feature 256j + 128i + p
    hsplit = []
    for b in range(B):
        ht = hidden_states[b].T.astype(np.float32)                 # [1024, S]
        hp = np.ascontiguousarray(
            ht.reshape(4, 2, 128, S).transpose(2, 0, 1, 3))        # [128,4,2,S]
        hsplit.append(_hilo(hp))
    in_maps = []
    for c in range(NCORES):
        b, g = divmod(c, 4)
        q_rows = w_qkv[256 * g: 256 * g + 256]
        k_rows = w_qkv[1024 + 64 * g: 1024 + 64 * g + 64]
        v_rows = w_qkv[1280 + 64 * g: 1280 + 64 * g + 64]
        wqk = np.concatenate([q_rows, k_rows, v_rows], axis=0)     # [384, 1024]
        # x16 into fp8 range; [p, rc, j, i, m] with h = 256j+128i+p
        wqkT = (wqk.T * 16.0).astype(np.float32)                   # [1024, 384]
        wpk = np.ascontiguousarray(
            wqkT.reshape(4, 2, 128, 3, 128).transpose(2, 3, 0, 1, 4))
        whi_a, wlo_a = _hilo(wpk)
        woTf = (w_o[:, 256 * g: 256 * g + 256].T * 16.0).astype(np.float32)
        wo_pk = np.ascontiguousarray(
            woTf.reshape(2, 128, HID).transpose(1, 0, 2))          # [128,2,HID]
        wohi_a, wolo_a = _hilo(wo_pk)
        in_maps.append(
            {
                "hhi": hsplit[b][0],
                "hlo": hsplit[b][1],
                "whi": whi_a,
                "wlo": wlo_a,
                "wohi": wohi_a,
                "wolo": wolo_a,
                "cosd": cosc,
                "sind": sinc,
            }
        )
    return in_maps


def _run(inputs, **spmd_kwargs):
    from concourse.bass_utils import run_bass_kernel_spmd

    nc = _get_nc()
    in_maps = _shard_inputs(**inputs)
    res = run_bass_kernel_spmd(
        nc, in_maps, core_ids=list(range(NCORES)), **spmd_kwargs
    )
    outs = []
    for b in range(B):
        acc = res.results[4 * b]["out"].astype(np.float32)
        for g in range(1, 4):
            acc = acc + res.results[4 * b + g]["out"].astype(np.float32)
        outs.append(acc * OUT_SCALE)
    return np.stack(outs, axis=0), res


def kernel(**inputs):
    out, _ = _run(inputs)
    return out



# revision 69
# speedup vs baseline: 1.0719x; 1.0087x over previous
"""GQA attention layer (QKV proj + RoPE + softmax attention + out proj) on 8
Trainium2 NeuronCores.

Sharding: core c = (batch b = c//4, head-group g = c%4): 4 q heads + 1 kv
head per core, w_o row-parallel partial output in bf16; the host upcasts,
sums the 4 partials per batch and divides by the fp8 scaling factor (512).

Design highlights vs the original baseline (234 us -> 164 us):
- fp8 hi/lo DoubleRow matmuls for both projections: operands split into
  e4m3 hi + e5m2 lo (host-side for h/w_qkv/w_o, Pool-side for the attention
  output); each K=256 block runs as 3 DR matmuls (hi*hi + hi*lo + lo*hi) at
  0.75x the bf16 PE cost with ~bf16 accuracy. scores/attnV stay bf16
  (single-fp8 would blow the 2e-2 error budget).
- SCALE folded into the exp activation so q and k share one compact rope
  table; exp runs on Activation in [128, 1024] tiles; 1-2 tiles per head
  (~18% of keys) offload to DVE via a Schraudolph bitcast exp
  (int16(A*s + B) reinterpreted as bf16), weighted toward rounds where the
  PE has no filler work.
- softmax normalize: pacc copied off PSUM immediately (the psA bank recycles
  in one op), reciprocal on a partition-0 staged denominator (PSUM-sourced
  or partition-offset reciprocal inputs return garbage on HW), Pool
  partition_broadcast + Pool multiply replace the baseline's fp32 PE
  broadcast matmuls (everything SBUF-side is Pool-legal).
- schedule: two staggered attention-head generators with a filler queue
  (projections at 2x pump rate, V transposes, outproj tiles at 1/4 rate to
  cover whole q-tiles); junk warm-up matmuls bridge the initial DMA wait and
  the final norm chain so the PE p-state never drops mid-kernel; DMA order
  tuned so each k/v tile lands just before the attention wavefront needs it.
"""

import collections

import numpy as np
import ml_dtypes

B, S, HID = 2, 2048, 1024
NH, NKV, D = 16, 4, 64
SCALE = float(D ** -0.5)
NCORES = 8
TT = 512          # token tile
NTT = S // TT     # 4
KC = S // 128     # 16 key chunks
NCP = KC // 2     # 8 chunk pairs
OUT_SCALE = 1.0 / 512.0

_BF16 = ml_dtypes.bfloat16
_E4 = ml_dtypes.float8_e4m3
_E5 = ml_dtypes.float8_e5m2

_nc_cache = None

# schedule knobs (tuned by simulation sweep; defaults are the shipped config)
CFG = {
    "transp_eng": "act",   # engine for the vaug drain after V transposes
    "q10_slot": "psA",     # PSUM slot for the prologue proj_q(1, 0)
    "dma_hlo_early": True,  # issue hlo tile0 before wlo2
    "warm_n": 22,           # prologue warm-up matmuls
    "ob_pat": "alt2",       # outproj PSUM-drain engine pattern
    "stagger": 12,          # rounds before the second head slot starts
    "exp_pat": "a",         # which chunk-pairs Schraudolph on DVE per head
    "pump_div": 2,          # outproj filler pump cadence (1 per N rounds)
    "keep_n": 6,            # attnV trails scores by keep_n chunk-pairs
    "q00_pbf_act": True,    # drain q00 PSUM on Act (idle in prologue)
    "tail_warm": 0,         # junk matmuls before the final outproj (unused)
    "finegrain_last": False,  # per-chunk finish for late heads (not a win)
    "wide_tail": True,      # tail outproj as 4 full-width tiles in psS
    "tail_dma_pool": True,  # alternate tail DMA issues between SP and Pool
    "split_last_ob": False,  # halve the final ob drain (not a win)
}


def _build_bass():
    import concourse.bass as bass
    import concourse.mybir as mybir
    import concourse.tile as tile
    from concourse import bacc
    from concourse.masks import make_identity

    BF = mybir.dt.bfloat16
    F32 = mybir.dt.float32
    I16 = mybir.dt.int16
    E4 = mybir.dt.float8e4
    E5 = mybir.dt.float8e5
    AF = mybir.ActivationFunctionType
    MULT = mybir.AluOpType.mult
    ADD = mybir.AluOpType.add
    SUB = mybir.AluOpType.subtract
    DR = mybir.MatmulPerfMode.DoubleRow
    # Schraudolph exp for offloaded tiles: bitcast(int16(s*A + B)) ~ exp(s)
    SCH_A = 184.66496280558537 * SCALE   # 128/ln2, scores carry 1/SCALE
    SCH_B = 16256.0 - 5.75 + 0.5         # bias center + truncation fix

    nc = bacc.Bacc()
    # h split hi/lo, packed [p, j, i, t]: h feature 256j + 128i + p
    hhi = nc.dram_tensor("hhi", (128, 4, 2, S), E4, kind="ExternalInput")
    hlo = nc.dram_tensor("hlo", (128, 4, 2, S), E5, kind="ExternalInput")
    # wqk split hi/lo, packed [p, rc, j, i, m]
    whi = nc.dram_tensor("whi", (128, 3, 4, 2, 128), E4, kind="ExternalInput")
    wlo = nc.dram_tensor("wlo", (128, 3, 4, 2, 128), E5, kind="ExternalInput")
    wohi = nc.dram_tensor("wohi", (128, 2, HID), E4, kind="ExternalInput")
    wolo = nc.dram_tensor("wolo", (128, 2, HID), E5, kind="ExternalInput")
    # shared q/k rope tables (/16); SCALE is applied by the exp activation
    cosd = nc.dram_tensor("cosd", (64, S), BF, kind="ExternalInput")
    sind = nc.dram_tensor("sind", (64, S), BF, kind="ExternalInput")
    out = nc.dram_tensor("out", (S, HID), BF, kind="ExternalOutput")

    with tile.TileContext(nc) as tc:
        with (
            tc.tile_pool(name="persist", bufs=1) as pp,
            tc.tile_pool(name="pbfp", bufs=3) as pbfp,
            tc.tile_pool(name="rope", bufs=3) as rp,
            tc.tile_pool(name="exps", bufs=CFG.get("ep_bufs", 6)) as ep,
            tc.tile_pool(name="norm", bufs=4) as np_,
            tc.tile_pool(name="outsb", bufs=4) as op_,
        ):
            # ---- persistent SBUF tiles + input loads, kv-first order
            hhi_sb = pp.tile([128, 4, 2, S], E4, tag="hhi_sb")
            hlo_sb = pp.tile([128, 4, 2, S], E5, tag="hlo_sb")
            whi_sb = pp.tile([128, 3, 4, 2, 128], E4, tag="whi_sb")
            wlo_sb = pp.tile([128, 3, 4, 2, 128], E5, tag="wlo_sb")
            wohi_sb = pp.tile([128, 2, HID], E4, tag="wohi_sb")
            wolo_sb = pp.tile([128, 2, HID], E5, tag="wolo_sb")
            # [128, S]: rows 0:64 DMA'd, rows 64:128 duplicated on-device
            cos_sb = pp.tile([128, S], BF, tag="cos_sb")
            sin_sb = pp.tile([128, S], BF, tag="sin_sb")

            def h_slice(t0, t1):
                for hd, hs in ((hhi, hhi_sb), (hlo, hlo_sb)):
                    nc.sync.dma_start(hs[:, :, :, t0:t1], hd[:, :, :, t0:t1])

            def cossin(tt):
                tts_ = bass.ts(tt, TT)
                nc.sync.dma_start(cos_sb[0:64, tts_], cosd[:, tts_])
                nc.sync.dma_start(sin_sb[0:64, tts_], sind[:, tts_])
                # Pool duplicates the tables onto partitions 64:128
                nc.gpsimd.tensor_copy(cos_sb[64:128, tts_],
                                      cos_sb[0:64, tts_])
                nc.gpsimd.tensor_copy(sin_sb[64:128, tts_],
                                      sin_sb[0:64, tts_])

            # prologue DMA order: hi-dtype pieces first so the interleaved
            # hi-step matmuls can start while the lo pieces stream in
            dma_ops = {
                "whi2": lambda: nc.sync.dma_start(whi_sb[:, 2], whi[:, 2]),
                "hhi02": lambda: nc.sync.dma_start(
                    hhi_sb[:, 0:2, :, 0:TT], hhi[:, 0:2, :, 0:TT]),
                "hhi24": lambda: nc.sync.dma_start(
                    hhi_sb[:, 2:4, :, 0:TT], hhi[:, 2:4, :, 0:TT]),
                "hlo": lambda: nc.sync.dma_start(
                    hlo_sb[:, :, :, 0:TT], hlo[:, :, :, 0:TT]),
                "wlo2": lambda: nc.sync.dma_start(wlo_sb[:, 2], wlo[:, 2]),
                "whi01": lambda: nc.sync.dma_start(
                    whi_sb[:, 0:2], whi[:, 0:2]),
                "wlo01": lambda: nc.sync.dma_start(
                    wlo_sb[:, 0:2], wlo[:, 0:2]),
                "cos0": lambda: cossin(0),
            }
            for tt in range(1, NTT):
                dma_ops[f"h{tt}"] = (
                    lambda tt=tt: h_slice(tt * TT, (tt + 1) * TT))
                dma_ops[f"cos{tt}"] = lambda tt=tt: cossin(tt)
            dma_ops["wo"] = lambda: (
                nc.sync.dma_start(wohi_sb[:], wohi[:]),
                nc.sync.dma_start(wolo_sb[:], wolo[:]))
            orders = {
                "v0": ["whi2", "hhi02", "hhi24", "hlo", "wlo2", "cos0",
                       "whi01", "wlo01",
                       "h1", "cos1", "h2", "cos2", "h3", "cos3", "wo"],
                "v6": ["whi2", "hhi02", "hhi24", "hlo", "wlo2", "cos0",
                       "h1", "whi01", "wlo01",
                       "cos1", "h2", "cos2", "h3", "cos3", "wo"],
                "v7": ["whi2", "hhi02", "hhi24", "hlo", "wlo2", "cos0",
                       "h1", "h2", "whi01", "wlo01",
                       "cos1", "cos2", "h3", "cos3", "wo"],
                "v8": ["whi2", "hhi02", "hhi24", "hlo", "wlo2", "cos0",
                       "whi01", "h1", "wlo01",
                       "cos1", "h2", "cos2", "h3", "cos3", "wo"],
                "v9": ["whi2", "hhi02", "hhi24", "hlo", "wlo2", "cos0",
                       "whi01", "wlo01",
                       "h1", "h2", "h3", "cos1", "cos2", "cos3", "wo"],
                "v10": ["whi2", "hhi02", "hhi24", "hlo", "wlo2", "cos0",
                        "whi01", "wlo01",
                        "h1", "h2", "cos1", "h3", "cos2", "cos3", "wo"],
            }
            for op in orders[CFG.get("dma_order", "v0")]:
                dma_ops[op]()

            ident = pp.tile([64, 64], BF, tag="ident")
            make_identity(nc, ident[:])
            ident128 = pp.tile([128, 128], BF, tag="ident128")
            make_identity(nc, ident128[:])
            warm = pp.tile([1, 8], F32, tag="warm")
            nc.any.memset(warm[:], 0.0)
            nc.scalar.activation(warm[:], warm[:], AF.Exp)
            # keep the tensor engine busy on junk matmuls while the first
            # h/w DMAs land, so the p-state ramp completes before real work
            wa = pp.tile([128, TT], BF, tag="wa")
            nc.gpsimd.memset(wa[:], 0.5)

            qrot = [pp.tile([128, S], BF, tag=f"qrot{p}", name=f"qrot{p}")
                    for p in range(2)]
            k2 = pp.tile([128, S], BF, tag="k2")
            vT = pp.tile([64, S], BF, tag="vT")
            vaug = pp.tile([128, KC, 65], BF, tag="vaug")
            nc.any.memset(vaug[:], 1.0 / 32.0)
            # anorm = 32*attn: bf16 full + fp8 hi/lo for the DR outproj,
            # packed [p, oc(=pair), t]
            anorm = pp.tile([128, 2, S], BF, tag="anorm")
            ahi = pp.tile([128, 2, S], E4, tag="ahi")
            alo = pp.tile([128, 2, S], E5, tag="alo")

            with (
                tc.tile_pool(name="psP", bufs=1, space="PSUM") as psP,
                tc.tile_pool(name="psS", bufs=3, space="PSUM") as psS,
                tc.tile_pool(name="psA", bufs=1, space="PSUM") as psA,
            ):

                def rope(pbf, dest, rows, tts, tag, eng=None):
                    """Pool builds the 32-block-swapped copy; the multiply/add
                    ops (all-SBUF bf16) default to DVE 2x mode but can run on
                    Pool when DVE is the busier engine."""
                    if eng is None:
                        eng = nc.vector
                    sg = rp.tile([128, TT], BF, tag=f"sg{tag}")
                    for blk in range(rows // 32):
                        src = blk ^ 1
                        nc.gpsimd.tensor_copy(
                            sg[32 * blk: 32 * blk + 32, :],
                            pbf[32 * src: 32 * src + 32, :])
                    t1 = rp.tile([128, TT], BF, tag=f"t1{tag}")
                    rt = rp.tile([128, TT], BF, tag=f"rt{tag}")
                    eng.tensor_tensor(
                        t1[0:rows, :], pbf[0:rows, :], cos_sb[0:rows, tts],
                        MULT)
                    eng.tensor_tensor(
                        rt[0:rows, :], sg[0:rows, :], sin_sb[0:rows, tts],
                        MULT)
                    eng.tensor_tensor(
                        dest, t1[0:rows, :], rt[0:rows, :], ADD)

                def proj_alloc(tt, name, slot):
                    tts = bass.ts(tt, TT)
                    pool, tag = slot
                    if tag == "sc":
                        ps = pool.tile([128, 2 * TT], F32, tag="sc",
                                       name=name)[:, 0:TT]
                    else:
                        ps = pool.tile([128, TT], F32, tag=tag, name=name)
                    return ps, tts

                def proj_steps(rc, tts, ps, phase=None):
                    """hi/lo DoubleRow projection: 12 accumulating DR matmuls
                    (4 K=256 blocks x {hi*hi, hi*lo, lo*hi}). phase="hi" emits
                    only the 4 hi*hi steps (start), "lo" the 8 mixed ones
                    (stop) -- used to interleave two DMA-gated projections in
                    the prologue."""
                    hi = [(whi_sb[:, rc, j], hhi_sb[:, j, :, tts])
                          for j in range(4)]
                    lo = []
                    for j in range(4):
                        lo.append((whi_sb[:, rc, j], hlo_sb[:, j, :, tts]))
                        lo.append((wlo_sb[:, rc, j], hhi_sb[:, j, :, tts]))
                    if phase == "hi":
                        steps, start, stop = hi, True, False
                    elif phase == "lo":
                        steps, start, stop = lo, False, True
                    else:
                        steps, start, stop = hi + lo, True, True
                    for si, (w, x) in enumerate(steps):
                        nc.tensor.matmul(
                            ps[:], w, x,
                            start=(start and si == 0),
                            stop=(stop and si == len(steps) - 1),
                            perf_mode=DR)

                def proj(rc, tt, name, slot):
                    ps, tts = proj_alloc(tt, name, slot)
                    proj_steps(rc, tts, ps)
                    return ps, tts

                def proj_kv_finish(ps, tts, tt, act_copy=False):
                    kbf = pbfp.tile([128, TT], BF, tag="pbf", name=f"kbf{tt}")
                    if act_copy:
                        nc.scalar.copy(kbf[0:64, :], ps[0:64, :])
                        nc.scalar.copy(vT[:, tts], ps[64:128, :])
                        nc.vector.tensor_scalar_mul(vT[:, tts], vT[:, tts],
                                                    1.0 / 16.0)
                    else:
                        nc.vector.tensor_copy(kbf[0:64, :], ps[0:64, :])
                        # v = ps/16
                        nc.vector.tensor_scalar_mul(vT[:, tts], ps[64:128, :],
                                                    1.0 / 16.0)
                    rope(kbf, k2[0:64, tts], 64, tts, "k")
                    nc.gpsimd.tensor_copy(k2[64:128, tts], k2[0:64, tts])

                def proj_kv(tt, slot=(psP, "proj"), act_copy=False):
                    ps, tts = proj(2, tt, f"projkv_{tt}", slot)
                    proj_kv_finish(ps, tts, tt, act_copy=act_copy)

                def transp(tt, slot=(psP, "proj")):
                    pool, tag = slot
                    pt = pool.tile([128, 4, 64], BF, tag=tag,
                                   name=f"vt{tt}")
                    for ci in range(4):
                        c = 4 * tt + ci
                        nc.tensor.transpose(pt[:, ci, :],
                                            vT[:, bass.ts(c, 128)], ident[:])
                    # Act drains vaug: these run in the prologue window where
                    # Act has no exp work yet
                    if CFG["transp_eng"] == "act":
                        nc.scalar.copy(vaug[:, 4 * tt: 4 * tt + 4, 0:64],
                                       pt[:])
                    else:
                        nc.vector.tensor_copy(
                            vaug[:, 4 * tt: 4 * tt + 4, 0:64], pt[:])

                def proj_q_finish(ps, tts, rc, tt):
                    pbf = pbfp.tile([128, TT], BF, tag="pbf",
                                    name=f"qbf{rc}_{tt}")
                    if tt == 0 and rc == 0 and CFG.get("q00_pbf_act", False):
                        nc.scalar.copy(pbf[:], ps[:])
                    else:
                        nc.vector.tensor_copy(pbf[:], ps[:])
                    # later q-tiles rope on Pool: they are pumped well ahead
                    # of their heads, and it keeps DVE free for PSUM drains
                    reng = nc.gpsimd if tt >= 1 else nc.vector
                    rope(pbf, qrot[rc][:, tts], 128, tts, "q", eng=reng)

                def proj_q(rc, tt, slot=(psP, "proj"), act_copy=False):
                    ps, tts = proj(rc, tt, f"projq{rc}_{tt}", slot)
                    proj_q_finish(ps, tts, rc, tt)

                def ob_drain(ob_sl, po_sl, dram_sl):
                    # PSUM drain: only Act/DVE may read PSUM; DVE is the
                    # cheaper copier so it takes most of them
                    pat = CFG["ob_pat"]
                    k = ob_ctr[0]
                    ob_ctr[0] += 1
                    if pat == "rr3":
                        on_act = k % 3 == 1
                    elif pat == "alt2":
                        on_act = k % 2 == 1
                    else:
                        on_act = False
                    if on_act:
                        nc.scalar.copy(ob_sl, po_sl)
                    else:
                        nc.vector.tensor_copy(ob_sl, po_sl)
                    nc.sync.dma_start(dram_sl, ob_sl)

                def outproj_wide(tch):
                    """Full-width tail outproj tile: po spans all of HID in
                    one 2-bank psS tile, so each tail tile costs one ob copy,
                    one DMA issue, and one whole-row (2KB-run) transfer."""
                    tcs = bass.ts(tch, 128)
                    po = psS.tile([128, 2 * TT], F32, tag="sc",
                                  name=f"pow{tch}")
                    # a matmul's output must stay within one PSUM bank
                    # (512 f32), so the wide tile accumulates in two halves
                    for ht in range(2):
                        hts = bass.ts(ht, TT)
                        terms = [(ahi[:, :, tcs], wohi_sb[:, :, hts]),
                                 (ahi[:, :, tcs], wolo_sb[:, :, hts]),
                                 (alo[:, :, tcs], wohi_sb[:, :, hts])]
                        for si, (a, w) in enumerate(terms):
                            nc.tensor.matmul(
                                po[:, hts], a, w,
                                start=(si == 0),
                                stop=(si == len(terms) - 1),
                                perf_mode=DR)
                    ob = op_.tile([128, 2 * TT], BF, tag="obw")
                    if tch == 4 * NTT - 1 and CFG.get("split_last_ob", False):
                        # final tile: drain + DMA in independent halves so
                        # the very last transfer starts as early as possible
                        nc.vector.tensor_copy(ob[:, 0:TT], po[:, 0:TT])
                        nc.sync.dma_start(out[tcs, 0:TT], ob[:, 0:TT])
                        nc.scalar.copy(ob[:, TT:], po[:, TT:])
                        nc.gpsimd.dma_start(out[tcs, TT:], ob[:, TT:])
                        return
                    k = ob_ctr[0]
                    ob_ctr[0] += 1
                    if k % 2 == 1:
                        nc.scalar.copy(ob[:], po[:])
                    else:
                        nc.vector.tensor_copy(ob[:], po[:])
                    # alternate the DMA issue engine so the four tail issues
                    # don't serialize on SP (Pool idles at the tail)
                    if CFG.get("tail_dma_pool", False) and tch % 2 == 1:
                        nc.gpsimd.dma_start(out[tcs, :], ob[:])
                    else:
                        nc.sync.dma_start(out[tcs, :], ob[:])

                def outproj_tile(tch, ht, last=False, slot=(psP, "proj"),
                                 split=False):
                    tcs = bass.ts(tch, 128)
                    pool, tag = slot
                    if tag == "sc":
                        po = pool.tile([128, 2 * TT], F32, tag="sc",
                                       name=f"po{tch}_{ht}")[:, 0:TT]
                    else:
                        po = pool.tile([128, TT], F32, tag=tag,
                                       name=f"po{tch}_{ht}")
                    ob = op_.tile([128, TT], BF, tag="ob")
                    # split=True emits two half-width accumulation groups so
                    # the drain+DMA pipeline runs at twice the granularity
                    # (same PE rows; used for the tail tiles)
                    nh = 2 if split else 1
                    for hh in range(nh):
                        h0 = ht * TT + hh * (TT // nh)
                        hts = slice(h0, h0 + TT // nh)
                        pos = slice(hh * (TT // nh), (hh + 1) * (TT // nh))
                        terms = [(ahi[:, :, tcs], wohi_sb[:, :, hts]),
                                 (ahi[:, :, tcs], wolo_sb[:, :, hts]),
                                 (alo[:, :, tcs], wohi_sb[:, :, hts])]
                        for si, (a, w) in enumerate(terms):
                            nc.tensor.matmul(
                                po[:, pos], a, w,
                                start=(si == 0), stop=(si == len(terms) - 1),
                                perf_mode=DR)
                        ob_drain(ob[:, pos], po[:, pos], out[tcs, hts])

                # ---- filler queue
                filler = collections.deque()
                ob_ctr = [0]

                def pump(n=1):
                    for _ in range(n):
                        if not filler:
                            return
                        filler.popleft()()

                def attention_head(pair, h2, qt, exp_map=None,
                                   last_head=False, keep_n=None,
                                   finegrain=False):
                    if exp_map is None:
                        exp_map = {}
                    if keep_n is None:
                        keep_n = CFG["keep_n"]
                    qts = bass.ts(qt, TT)
                    qrows = slice(64 * h2, 64 * h2 + 64)
                    pacc = psA.tile([128, 4, 65], F32, tag="att",
                                    name=f"att{pair}_{h2}_{qt}")
                    pending = collections.deque()

                    def drain_pending(keep):
                        # transposed attnV: the ex chunk is the STATIONARY
                        # operand and V+ones the moving one -> out free is 65
                        # (attn^T per 128-token chunk, denominator in col 64)
                        while len(pending) > keep:
                            pex, pcp = pending.popleft()
                            for j in range(2):
                                c = 2 * pcp + j
                                for qc in range(4):
                                    # the 4 qc accumulation groups share one
                                    # PSUM zero region: start/stop only once
                                    nc.tensor.matmul(
                                        pacc[:, qc, :],
                                        pex[:, 512 * j + 128 * qc:
                                            512 * j + 128 * qc + 128],
                                        vaug[:, c, :],
                                        start=(c == 0 and qc == 0),
                                        stop=(c == KC - 1 and qc == 3))

                    for cp in range(NCP):
                        sc = psS.tile([128, 2 * TT], F32, tag="sc",
                                      name=f"sc{pair}_{h2}_{qt}_{cp}")
                        for j in range(2):
                            c = 2 * cp + j
                            nc.tensor.matmul(
                                sc[:, bass.ts(j, TT)],
                                k2[qrows, bass.ts(c, 128)],
                                qrot[pair][qrows, qts],
                                start=True, stop=True,
                                tile_position=(64 * h2, 0))
                        if exp_map.get(cp) == "dve":
                            # offload this tile's exp to DVE (Schraudolph);
                            # ~2% approx error on those keys is within budget
                            # and relieves the pacing Activation engine.
                            # (Pool cannot read PSUM, so only DVE can assist.)
                            exi = ep.tile([128, 2 * TT], I16, tag="exps")
                            nc.vector.tensor_scalar(exi[:], sc[:],
                                                    SCH_A, SCH_B, MULT, ADD)
                            ex = exi[:].bitcast(BF)
                        else:
                            ext = ep.tile([128, 2 * TT], BF, tag="exp")
                            nc.scalar.activation(ext[:], sc[:], AF.Exp,
                                                 scale=SCALE)
                            ex = ext[:]
                        pending.append((ex, cp))
                        # attnV trails scores by two cpairs so the exp
                        # semaphore has always fired by the time the PE
                        # reaches the accumulation matmuls
                        drain_pending(keep_n)
                        yield
                    drain_pending(0)
                    # attn^T normalize: the denominator is per-PARTITION ->
                    # Pool normalize_recip does the whole softmax divide;
                    # PE transposes back to [o, t] for the outproj
                    att = np_.tile([128, 4, 65], F32, tag="att_sb")
                    nrm = np_.tile([128, 4, 64], BF, tag="nrm")
                    # the last heads' transpose borrows the freed psA bank so
                    # the tail outproj tiles never queue behind it on psP
                    atp_on_a = last_head and CFG.get("atp_psa", True)
                    atp_pool, atp_tag = (psA, "att") if atp_on_a \
                        else (psP, "proj")
                    pt = atp_pool.tile([64, 4, 128], BF, tag=atp_tag,
                                       name=f"atp{pair}_{h2}_{qt}")
                    fine = finegrain and CFG.get("finegrain_last", True)
                    if fine:
                        # per-token-chunk finish: tail outproj tile tch only
                        # needs chunk (tch - 12) of each head, so chunk 0's
                        # chain completing early unblocks po12 right away
                        for qc in range(4):
                            qcs = slice(qt * TT + 128 * qc,
                                        qt * TT + 128 * qc + 128)
                            nc.vector.tensor_copy(att[:, qc, :],
                                                  pacc[:, qc, :])
                            nc.gpsimd.normalize_recip(
                                nrm[:, qc, :], att[:, qc, 0:64],
                                att[:, qc, 64:65])
                            nc.tensor.transpose(pt[:, qc, :], nrm[:, qc, :],
                                                ident128[:])
                            nc.vector.tensor_copy(anorm[qrows, pair, qcs],
                                                  pt[:, qc, :])
                            nc.gpsimd.tensor_copy(ahi[qrows, pair, qcs],
                                                  anorm[qrows, pair, qcs])
                            nc.gpsimd.tensor_tensor(
                                alo[qrows, pair, qcs],
                                anorm[qrows, pair, qcs],
                                ahi[qrows, pair, qcs],
                                SUB)
                        return
                    nc.vector.tensor_copy(att[:], pacc[:])
                    for qc in range(4):
                        nc.gpsimd.normalize_recip(
                            nrm[:, qc, :], att[:, qc, 0:64],
                            att[:, qc, 64:65])
                    for qc in range(4):
                        nc.tensor.transpose(pt[:, qc, :], nrm[:, qc, :],
                                            ident128[:])
                    nc.vector.tensor_copy(anorm[qrows, pair, qts], pt[:])
                    # fp8 hi/lo for the DR outproj on Pool (all-SBUF; Pool is
                    # the idle engine at the tail)
                    nc.gpsimd.tensor_copy(ahi[qrows, pair, qts],
                                          anorm[qrows, pair, qts])
                    nc.gpsimd.tensor_tensor(
                        alo[qrows, pair, qts],
                        anorm[qrows, pair, qts],
                        ahi[qrows, pair, qts],
                        SUB)

                # ---- master schedule: kv0/q00 up front with Activation-
                # assisted copies (exp idle), V-transpose 0 borrows the psA
                # ring; later h tiles are DMA-gated so they pump as filler.
                def warm_mms(n, label):
                    for wi in range(n):
                        wps = psA.tile([128, TT], F32, tag="att",
                                       name=f"warm{label}_{wi}")
                        nc.tensor.matmul(wps[:, 0:256], wa[:, 0:128],
                                         wa[:, 0:256], start=True, stop=True)

                warm_mms(CFG["warm_n"], "a")
                # prologue projections spread across the still-idle psS banks
                # so consecutive projections never serialize on one PSUM bank.
                # kv0 and q00 interleave their hi and lo step groups: the hi
                # steps only need the hi-dtype DMAs (which land first), so the
                # PE isn't stuck in-order behind kv0's lo steps waiting on hlo
                if CFG.get("interleave_prologue", True):
                    ps_kv, tts0 = proj_alloc(0, "projkv_0", (psP, "proj"))
                    ps_q, _ = proj_alloc(0, "projq0_0", (psS, "sc"))
                    proj_steps(2, tts0, ps_kv, "hi")
                    proj_steps(0, tts0, ps_q, "hi")
                    proj_steps(2, tts0, ps_kv, "lo")
                    proj_steps(0, tts0, ps_q, "lo")
                    proj_kv_finish(ps_kv, tts0, 0, act_copy=True)
                    proj_q_finish(ps_q, tts0, 0, 0)
                else:
                    proj_kv(0, slot=(psP, "proj"), act_copy=True)
                    proj_q(0, 0, slot=(psS, "sc"), act_copy=True)
                transp(0, slot=(psA, "att"))
                proj_kv(1, slot=(psS, "sc"))
                q10_slot = ((psS, "sc") if CFG["q10_slot"] == "psS"
                            else (psA, "att"))
                filler.append(lambda: proj_kv(2, slot=(psS, "sc")))
                filler.append(lambda: transp(1))
                filler.append(lambda: proj_q(1, 0, slot=q10_slot))
                filler.append(lambda: proj_kv(3))
                filler.append(lambda: transp(2))
                filler.append(lambda: transp(3))
                for tt in range(1, NTT):
                    for rc in range(2):
                        filler.append(
                            lambda rc=rc, tt=tt: proj_q(rc, tt))

                heads = [(pair, h2, qt)
                         for qt in range(NTT)
                         for pair in range(2)
                         for h2 in range(2)]

                # tail (qt3) tiles rotate over 4 PSUM slots (psP + the three
                # psS bufs, idle once scores end) so po matmuls never wait on
                # an ob drain: with 4 slots in flight the ~600ns copy hides
                # behind 4 x 320ns of matmuls
                tail_ctr = [0]

                def tail_slot():
                    s = ((psP, "proj") if tail_ctr[0] % 4 == 0
                         else (psS, "sc"))
                    tail_ctr[0] += 1
                    return s

                def head_done(i):
                    if i % 4 == 3:
                        qt = heads[i][2]
                        last = i == len(heads) - 1
                        for tch in range(4 * qt, 4 * qt + 4):
                            if qt == 3 and CFG.get("wide_tail", False):
                                filler.append(
                                    lambda tch=tch: outproj_wide(tch))
                                continue
                            for ht in range(2):
                                if qt == 3:
                                    filler.append(
                                        lambda tch=tch, ht=ht, last=last:
                                        outproj_tile(
                                            tch, ht, last=last,
                                            slot=tail_slot(),
                                            split=CFG.get("split_tail",
                                                          False)))
                                else:
                                    filler.append(
                                        lambda tch=tch, ht=ht, last=last:
                                        outproj_tile(tch, ht, last=last))

                nxt = 0
                # per-head exp engine map: Act is exact, DVE runs the
                # Schraudolph approximation. Act is the cheaper exp engine,
                # so DVE only takes what evens out the Act/DVE totals; the
                # middle heads get one extra (that is where the exp
                # backpressure stalls showed).
                def exp_map_for(i):
                    pat = CFG["exp_pat"]
                    if pat == "a":
                        if i == 0:
                            return {6: "dve"}
                        return {1: "dve", 3: "dve", 6: "dve"}
                    if pat == "b":
                        if i == 0:
                            return {6: "dve"}
                        if 4 <= i <= 11:
                            return {1: "dve", 3: "dve", 6: "dve"}
                        return {3: "dve", 6: "dve"}
                    if pat == "c":
                        return {1: "dve", 3: "dve", 6: "dve"}
                    if pat == "d":
                        if i == 0:
                            return {6: "dve"}
                        return {1: "dve", 4: "dve", 6: "dve"}
                    if pat == "e":
                        if i == 0:
                            return {6: "dve"}
                        return {2: "dve", 4: "dve", 6: "dve"}
                    if pat == "f":
                        if i == 0:
                            return {6: "dve"}
                        if i >= len(heads) - 2:
                            return {1: "dve", 3: "dve", 5: "dve", 6: "dve"}
                        return {1: "dve", 3: "dve", 6: "dve"}
                    if pat == "g":
                        if i == 0:
                            return {3: "dve", 6: "dve"}
                        return {1: "dve", 3: "dve", 5: "dve", 6: "dve"}
                    if pat == "h":
                        if i == 0:
                            return {6: "dve"}
                        if 4 <= i <= 11:
                            return {1: "dve", 3: "dve", 5: "dve", 6: "dve"}
                        return {1: "dve", 3: "dve", 6: "dve"}
                    raise ValueError(pat)

                def start_next():
                    nonlocal nxt
                    if nxt >= len(heads):
                        return None
                    is_late = nxt >= len(heads) - 2
                    em = dict(exp_map_for(nxt))
                    if is_late and CFG.get("last_cp7_dve", False):
                        em[7] = "dve"
                        em.pop(6, None)  # keep the approx-tile count level
                    g = attention_head(*heads[nxt],
                                       exp_map=em,
                                       last_head=(nxt == len(heads) - 1),
                                       keep_n=(CFG.get("keep_last")
                                               if is_late else None),
                                       finegrain=is_late)
                    nxt += 1
                    return (nxt - 1, g)

                nproj_fill = len(filler)
                n_slots = CFG.get("n_slots", 2)
                slots = [start_next()] + [None] * (n_slots - 1)
                stagger = CFG["stagger"]
                next_slot = 1
                countdown = stagger
                step = 0
                while any(slots):
                    for si in range(n_slots):
                        if slots[si] is None:
                            continue
                        i, g = slots[si]
                        try:
                            next(g)
                            # projection fillers drain at double rate (their
                            # PSUM->rope chains must stay ahead); outproj
                            # fillers at half rate so they cover the whole
                            # q-tile's rounds instead of bunching
                            if step < nproj_fill:
                                pump(2)
                            elif step % CFG["pump_div"] == 0:
                                pump(1)
                            step += 1
                            if next_slot < n_slots:
                                countdown -= 1
                                if countdown == 0:
                                    slots[next_slot] = start_next()
                                    next_slot += 1
                                    countdown = stagger
                        except StopIteration:
                            head_done(i)
                            slots[si] = start_next()
                # (no tail warm bridge needed: the cost model's p-state never
                # downclocks once ramped, so idle before the final outproj
                # tiles is free)
                if CFG.get("tail_warm", 0):
                    filler.appendleft(
                        lambda: warm_mms(CFG["tail_warm"], "t"))
                while filler:
                    pump(1)
    nc.finalize()
    return nc


def _get_nc():
    global _nc_cache
    if _nc_cache is None:
        _nc_cache = _build_bass()
    return _nc_cache


def _hilo(x):
    hi = x.astype(_E4)
    lo = (x - hi.astype(np.float32)).astype(_E5)
    return hi, lo


def _shard_inputs(hidden_states, cos, sin, w_qkv, w_o):
    """Build per-core input maps. Core c = (b = c // 4, g = c % 4)."""
    cosT = cos.T.astype(np.float32)                                # [64, S]
    sinT = sin.T.astype(np.float32)
    sinmod = np.concatenate([-sinT[0:32], sinT[32:64]], axis=0)
    cosc = np.ascontiguousarray(cosT / 16.0).astype(_BF16)
    sinc = np.ascontiguousarray(sinmod / 16.0).astype(_BF16)

    # h packed [p, j, i, t]: feature 256j + 128i + p
    hsplit = []
    for b in range(B):
        ht = hidden_states[b].T.astype(np.float32)                 # [1024, S]
        hp = np.ascontiguousarray(
            ht.reshape(4, 2, 128, S).transpose(2, 0, 1, 3))        # [128,4,2,S]
        hsplit.append(_hilo(hp))
    in_maps = []
    for c in range(NCORES):
        b, g = divmod(c, 4)
        q_rows = w_qkv[256 * g: 256 * g + 256]
        k_rows = w_qkv[1024 + 64 * g: 1024 + 64 * g + 64]
        v_rows = w_qkv[1280 + 64 * g: 1280 + 64 * g + 64]
        wqk = np.concatenate([q_rows, k_rows, v_rows], axis=0)     # [384, 1024]
        # x16 into fp8 range; [p, rc, j, i, m] with h = 256j+128i+p
        wqkT = (wqk.T * 16.0).astype(np.float32)                   # [1024, 384]
        wpk = np.ascontiguousarray(
            wqkT.reshape(4, 2, 128, 3, 128).transpose(2, 3, 0, 1, 4))
        whi_a, wlo_a = _hilo(wpk)
        woTf = (w_o[:, 256 * g: 256 * g + 256].T * 16.0).astype(np.float32)
        wo_pk = np.ascontiguousarray(
            woTf.reshape(2, 128, HID).transpose(1, 0, 2))          # [128,2,HID]
        wohi_a, wolo_a = _hilo(wo_pk)
        in_maps.append(
            {
                "hhi": hsplit[b][0],
                "hlo": hsplit[b][1],
                "whi": whi_a,
                "wlo": wlo_a,
                "wohi": wohi_a,
                "wolo": wolo_a,
                "cosd": cosc,
                "sind": sinc,
            }
        )
    return in_maps


def _run(inputs, **spmd_kwargs):
    from concourse.bass_utils import run_bass_kernel_spmd

    nc = _get_nc()
    in_maps = _shard_inputs(**inputs)
    res = run_bass_kernel_spmd(
        nc, in_maps, core_ids=list(range(NCORES)), **spmd_kwargs
    )
    outs = []
    for b in range(B):
        acc = res.results[4 * b]["out"].astype(np.float32)
        for g in range(1, 4):
            acc = acc + res.results[4 * b + g]["out"].astype(np.float32)
        outs.append(acc * OUT_SCALE)
    return np.stack(outs, axis=0), res


def kernel(**inputs):
    out, _ = _run(inputs)
    return out

